# revision 12
# baseline (speedup 1.0000x reference)
"""Trainium2 Bass kernel for nn_LocalRefinementUnit (KNN local refinement).

Sharding: 8 cores = (batch b = core//2) x (half h = core%2 of the 4096 points).
All candidate-side arrays (B5, recs, garr) are kept in GLOBAL point order, so
the two cores of a pair hold identical candidate state and only the query-side
inputs differ (no host-side rolls).

Single merged device program (the axon tunnel quantizes round trips to ~40ms,
so fewer dispatches beat phase-overlap tricks):
  setup:   B5/A5q from coords, recs = [h|dW] records, garr = W2a^T fe (int8
           features dequantized on device, scale folded into W2a)
  A+B1:    per 128-query chunk: -d2 via PE matmul, exact top-16, record
           gathers, delta-h (bf16), moment accumulation in PSUM
  AR1  ->  BN1 stats; BN3 stats via pinv(W1) trick; r1 = relu(dh^T+c1) in SBUF
  B2:      z2^T = W2b1^T r1 + G^T per chunk (G gathered once into SBUF),
           bn_stats; AR2 -> BN2 fold
  C:       z2' rebuild, relu*w_diag, PE transpose-accum -> weighted (fp16 out)
Residual add with full-precision F_E happens on host (threaded).
"""
import numpy as np
from concurrent.futures import ThreadPoolExecutor

import concourse.bass as bass
import concourse.mybir as mybir
import concourse.tile as tile
from concourse import bacc
from concourse.masks import make_identity

f32 = mybir.dt.float32
f16 = mybir.dt.float16
bf = mybir.dt.bfloat16
u32 = mybir.dt.uint32
i8 = mybir.dt.int8
AF = mybir.ActivationFunctionType

B, C, K = 4, 128, 16
EPS = 1e-5
N_CORES = 8
REC = 128          # record elems (f32): [h 64 | dW 16 | pad 48] = 512B


def build_full(rn=4096, n_cores=N_CORES):
    half = rn // 2
    nch = half // 128           # query chunks of 128
    nsc = rn // 128             # candidate chunks of 128 points
    ntot = n_cores * half * K   # global BN row count

    nc = bacc.Bacc("TRN2", target_bir_lowering=False, debug=False,
                   num_devices=n_cores, enable_asserts=False)

    q3c = nc.dram_tensor("q3c", [3, rn], f32, kind="ExternalInput").ap()
    q3q = nc.dram_tensor("q3q", [3, half], f32, kind="ExternalInput").ap()
    fe8 = nc.dram_tensor("fe8", [C, rn], i8, kind="ExternalInput").ap()
    # wpk[128,284]: [:,0:128]=W2a.T; [:,128:192]=W2b.T transposed ([C,64]);
    # [:,192]=g2; [:,193]=be2; [:,194]=int8 scale; rows0-63:
    # [:,196:276]=[W1.T|Ww.T] (rows0-2); [:,276:279]=pinv(W1).T;
    # [:,279]=g1; [:,280]=be1; [:,281] rows0-15=gw; [:,282]=bew
    wpk = nc.dram_tensor("wpk", [128, 284], f32, kind="ExternalInput").ap()

    y16 = nc.dram_tensor("y16", [C, half], f16, kind="ExternalOutput").ap()

    recs = nc.dram_tensor("recs", [rn, REC], f32).ap()
    garr = nc.dram_tensor("garr", [rn, C], bf).ap()
    ar1i = nc.dram_tensor("ar1i", [64, 65], f32).ap()
    ar1o = nc.dram_tensor("ar1o", [64, 65], f32, addr_space="Shared").ap()
    ar2i = nc.dram_tensor("ar2i", [C, 2], f32).ap()
    ar2o = nc.dram_tensor("ar2o", [C, 2], f32, addr_space="Shared").ap()
    rg = [list(range(n_cores))]

    with tile.TileContext(nc) as tc:
        with tc.tile_pool(name="persist", bufs=1) as pp, \
             tc.tile_pool(name="ppsum", bufs=1, space="PSUM") as ppp:
            ident = pp.tile([128, 128], f32)
            make_identity(nc, ident[:])
            ident_bf = pp.tile([128, 128], bf)
            nc.vector.tensor_copy(out=ident_bf[:], in_=ident[:])
            ones128 = pp.tile([128, 1], f32)
            nc.vector.memset(ones128[:], 1.0)
            onesrow_bf = pp.tile([1, 128], bf)
            nc.vector.memset(onesrow_bf[:], 1.0)

            wpk_sb = pp.tile([128, 284], f32)
            nc.sync.dma_start(wpk_sb[:], wpk[:])
            w1ww_sb = wpk_sb[0:3, 196:276]
            gpv_sb = wpk_sb[0:64, 276:279]
            g1c = wpk_sb[0:64, 279:280]
            be1c = wpk_sb[0:64, 280:281]
            gwc = wpk_sb[0:K, 281:282]
            bewc = wpk_sb[0:K, 282:283]
            g2c = wpk_sb[:, 192:193]
            be2c = wpk_sb[:, 193:194]

            fe8_sb = pp.tile([C, rn], i8)
            nc.sync.dma_start(fe8_sb[:], fe8[:])
            fe_sb = pp.tile([C, rn], f16)
            nc.vector.tensor_copy(out=fe_sb[:], in_=fe8_sb[:])
            w2at_f = pp.tile([128, 128], f32)
            nc.vector.tensor_mul(out=w2at_f[:], in0=wpk_sb[:, 0:128],
                                 in1=wpk_sb[:, 194:195].broadcast_to([128, 128]))
            w2at16 = pp.tile([128, 128], f16)
            nc.scalar.copy(out=w2at16[:], in_=w2at_f[:])

            # B5 = [q; 1; -sq] (candidates), A5q = [2q; -sq; 1] (queries)
            B5 = pp.tile([5, rn], f32)
            A5q = pp.tile([5, half], f32)
            q3q_sb = pp.tile([3, half], f32)
            nc.sync.dma_start(B5[0:3, :], q3c[:])
            nc.sync.dma_start(q3q_sb[:], q3q[:])

            idx_all = pp.tile([128, nch * K], u32)
            wdiff_all = pp.tile([128, nch * K], f32)
            wdp_all = pp.tile([128, nch * K], f32)
            mh_g = pp.tile([64, 65], f32)
            s1 = pp.tile([64, 1], f32)
            c1 = pp.tile([64, 1], f32)
            bn_all = pp.tile([128, nch * 4 * 6], f32)
            r1_sb = pp.tile([64, nch * K * 128], bf)
            w2bt = pp.tile([64, C], f32)
            w2bt1 = pp.tile([64, C], f32)
            w2bt1_bf = pp.tile([64, C], bf)
            w2bt2 = pp.tile([64, C], f32)
            w2bt2_bf = pp.tile([64, C], bf)
            c2row = pp.tile([1, C], f32)
            c2row_bf = pp.tile([1, C], bf)
            s2rep = pp.tile([C, C], f32)
            s2rep_bf = pp.tile([C, C], bf)
            ps_mh = ppp.tile([64, 65], f32, space="PSUM")

            pswt = ppp.tile([64, 128], f32, space="PSUM")
            nc.tensor.matmul(out=pswt[:], lhsT=wpk_sb[:, 128:192], rhs=ident[:],
                             is_transpose=True, start=True, stop=True)
            nc.scalar.copy(out=w2bt[:], in_=pswt[:])

            # ---------- setup: squared norms + records + garr ----------
            with tc.tile_pool(name="su", bufs=1) as su, \
                 tc.tile_pool(name="su2", bufs=2) as su2, \
                 tc.tile_pool(name="sup", bufs=2, space="PSUM") as sup:
                ones3 = su.tile([3, 1], f32, tag="ones3")
                nc.vector.memset(ones3[:], 1.0)
                onesr = su.tile([1, rn], f32, tag="onesr")
                nc.vector.memset(onesr[:], 1.0)
                nsqr = su.tile([1, rn], f32, tag="nsqr")
                q3sq = su.tile([3, rn], f32, tag="q3sq")
                nc.scalar.activation(out=q3sq[:], in_=B5[0:3, :], func=AF.Square)
                for i in range(rn // 512):
                    pssq = sup.tile([1, 512], f32, tag="pssq", space="PSUM")
                    nc.tensor.matmul(out=pssq[:], lhsT=ones3[:],
                                     rhs=q3sq[:, i * 512:(i + 1) * 512],
                                     start=True, stop=True)
                    nc.scalar.mul(out=nsqr[:, i * 512:(i + 1) * 512], in_=pssq[:],
                                  mul=-1.0)
                nc.sync.dma_start(B5[3:4, :], onesr[:])
                nc.sync.dma_start(B5[4:5, :], nsqr[:])
                nsqq = su.tile([1, half], f32, tag="nsqq")
                qqsq = su.tile([3, half], f32, tag="qqsq")
                nc.scalar.activation(out=qqsq[:], in_=q3q_sb[:], func=AF.Square)
                for i in range(half // 512):
                    psq = sup.tile([1, 512], f32, tag="pssq", space="PSUM")
                    nc.tensor.matmul(out=psq[:], lhsT=ones3[:],
                                     rhs=qqsq[:, i * 512:(i + 1) * 512],
                                     start=True, stop=True)
                    nc.scalar.mul(out=nsqq[:, i * 512:(i + 1) * 512], in_=psq[:],
                                  mul=-1.0)
                nc.scalar.mul(out=A5q[0:3, :], in_=q3q_sb[:], mul=2.0)
                nc.sync.dma_start(A5q[3:4, :], nsqq[:])
                nc.sync.dma_start(A5q[4:5, :], onesr[:, 0:half])
                for i in range(nsc):
                    sl = slice(i * 128, (i + 1) * 128)
                    psh = sup.tile([128, 80], f32, tag="psh", space="PSUM")
                    nc.tensor.matmul(out=psh[:], lhsT=B5[0:3, sl],
                                     rhs=w1ww_sb, start=True, stop=True)
                    hsb = su2.tile([128, 80], f32, tag="hsb")
                    nc.scalar.copy(out=hsb[:], in_=psh[:])
                    nc.sync.dma_start(recs[sl, 0:80], hsb[:])
                    psg = sup.tile([128, C], f32, tag="psg", space="PSUM")
                    nc.tensor.matmul(out=psg[:], lhsT=fe_sb[:, sl],
                                     rhs=w2at16[:], start=True, stop=True)
                    gsb = su2.tile([128, C], bf, tag="gsb")
                    nc.scalar.copy(out=gsb[:], in_=psg[:])
                    nc.sync.dma_start(garr[sl, :], gsb[:])

            # ---------- phase A + B1 ----------
            with tc.tile_pool(name="dhp", bufs=1) as dhp:
                dh_all = dhp.tile([128, nch * K * 65], bf)
                nc.vector.memset(
                    dh_all[:].rearrange("p (g o) -> p g o", o=65)[:, :, 64:65], 1.0)
                with tc.tile_pool(name="a1", bufs=1) as a1, \
                     tc.tile_pool(name="a2", bufs=2) as a2, \
                     tc.tile_pool(name="ap2", bufs=2, space="PSUM") as ap2:
                    for ci in range(nch):
                        qsl = slice(ci * 128, (ci + 1) * 128)
                        vals = a1.tile([128, rn], f32, tag="vals")
                        qw = min(1024, rn)
                        for qd in range(rn // qw):
                            psd = ap2.tile([128, qw], f32, tag="psd", space="PSUM")
                            for hh in range(qw // 512):
                                nc.tensor.matmul(
                                    out=psd[:, hh * 512:(hh + 1) * 512],
                                    lhsT=A5q[:, qsl],
                                    rhs=B5[:, qd * qw + hh * 512:qd * qw + (hh + 1) * 512],
                                    start=True, stop=True)
                            nc.scalar.copy(out=vals[:, qd * qw:qd * qw + 512],
                                           in_=psd[:, 0:512])
                            if qw > 512:
                                nc.scalar.copy(out=vals[:, qd * qw + 512:(qd + 1) * qw],
                                               in_=psd[:, 512:1024])
                        nseg = 16
                        sv = a2.tile([128, nseg * 8], f32, tag="sv")
                        for sgi in range(nseg):
                            nc.vector.max(out=sv[:, sgi * 8:(sgi + 1) * 8],
                                          in_=vals[:, sgi * (rn // 16):(sgi + 1) * (rn // 16)])
                        m1 = a2.tile([128, 8], f32, tag="m1")
                        m2 = a2.tile([128, 8], f32, tag="m2")
                        sv2 = a2.tile([128, nseg * 8], f32, tag="sv2")
                        nc.vector.max(out=m1[:], in_=sv[:])
                        nc.vector.match_replace(out=sv2[:], in_to_replace=m1[:],
                                                in_values=sv[:], imm_value=-1e30)
                        nc.vector.max(out=m2[:], in_=sv2[:])
                        nc.vector.max_index(out=idx_all[:, ci * K:ci * K + 8],
                                            in_max=m1[:], in_values=vals[:])
                        nc.vector.max_index(out=idx_all[:, ci * K + 8:ci * K + 16],
                                            in_max=m2[:], in_values=vals[:])

                        G = a2.tile([128, K, REC], f32, tag="G")
                        for k in range(K):
                            nc.gpsimd.indirect_dma_start(
                                out=G[:, k, :], out_offset=None, in_=recs[:],
                                in_offset=bass.IndirectOffsetOnAxis(
                                    ap=idx_all[:, ci * K + k:ci * K + k + 1], axis=0))
                        psh = ap2.tile([128, 80], f32, tag="psh2", space="PSUM")
                        nc.tensor.matmul(out=psh[:], lhsT=q3q_sb[:, qsl],
                                         rhs=w1ww_sb, start=True, stop=True)
                        hq = a2.tile([128, 80], f32, tag="hq")
                        nc.scalar.copy(out=hq[:], in_=psh[:])
                        dh_ci = dh_all[:, ci * K * 65:(ci + 1) * K * 65].rearrange(
                            "p (k j) -> p k j", k=K)[:, :, 0:64]
                        nc.vector.tensor_sub(out=dh_ci, in0=G[:, :, 0:64],
                                             in1=hq[:, 0:64].rearrange("p (o j) -> p o j", o=1).broadcast_to([128, K, 64]))
                        Gflat = G[:].rearrange("p k r -> p (k r)")
                        nc.vector.tensor_sub(out=wdiff_all[:, ci * K:(ci + 1) * K],
                                             in0=Gflat[:, 64:64 + 129 * (K - 1) + 1:129],
                                             in1=hq[:, 64:80])
                        for k in range(K):
                            base = ci * K * 65 + k * 65
                            dsl = dh_all[:, base:base + 64]
                            dsl65 = dh_all[:, base:base + 65]
                            st = (ci == 0 and k == 0)
                            sp = (ci == nch - 1 and k == K - 1)
                            nc.tensor.matmul(out=ps_mh[:], lhsT=dsl, rhs=dsl65,
                                             start=st, stop=sp, skip_group_check=True)

                # ---------- AR1 + BN1/BN3 stat folding ----------
                with tc.tile_pool(name="st", bufs=1) as st, \
                     tc.tile_pool(name="stp", bufs=2, space="PSUM") as stp:
                    mh_sb = st.tile([64, 65], f32)
                    nc.scalar.copy(out=mh_sb[:], in_=ps_mh[:])
                    nc.sync.dma_start(ar1i[:], mh_sb[:])
                    nc.gpsimd.collective_compute(
                        "AllReduce", mybir.AluOpType.add,
                        ins=[ar1i[:]], outs=[ar1o[:]], replica_groups=rg)
                    nc.sync.dma_start(mh_g[:], ar1o[:])

                    mud = st.tile([64, 1], f32)
                    nc.vector.tensor_scalar_mul(mud[:], mh_g[:, 64:65], 1.0 / ntot)
                    mask = st.tile([64, 64], f32)
                    nc.vector.tensor_mul(out=mask[:], in0=mh_g[:, 0:64],
                                         in1=ident[0:64, 0:64])
                    psd1 = stp.tile([64, 1], f32, tag="stsc", space="PSUM")
                    nc.tensor.matmul(out=psd1[:], lhsT=mask[:], rhs=ones128[0:64, :],
                                     start=True, stop=True)
                    var1 = st.tile([64, 1], f32)
                    nc.scalar.mul(out=var1[:], in_=psd1[:], mul=1.0 / ntot)
                    musq = st.tile([64, 1], f32)
                    nc.scalar.activation(out=musq[:], in_=mud[:], func=AF.Square)
                    nc.vector.tensor_sub(out=var1[:], in0=var1[:], in1=musq[:])
                    rs1 = st.tile([64, 1], f32)
                    nc.vector.tensor_scalar_add(var1[:], var1[:], EPS)
                    nc.scalar.activation(out=rs1[:], in_=var1[:], func=AF.Sqrt)
                    nc.vector.reciprocal(out=rs1[:], in_=rs1[:])
                    nc.vector.tensor_mul(out=s1[:], in0=rs1[:], in1=g1c)
                    inv1 = st.tile([64, 1], f32)
                    nc.vector.reciprocal(out=inv1[:], in_=s1[:])
                    nc.vector.tensor_mul(out=inv1[:], in0=inv1[:], in1=be1c)
                    nc.vector.tensor_sub(out=c1[:], in0=inv1[:], in1=mud[:])
                    nc.vector.tensor_mul(out=w2bt1[:], in0=w2bt[:],
                                         in1=s1[:].broadcast_to([64, C]))
                    nc.scalar.copy(out=w2bt1_bf[:], in_=w2bt1[:])

                    # BN3 via pinv: M3 = G Mh G^T
                    psp1 = stp.tile([3, 64], f32, tag="stsc", space="PSUM")
                    nc.tensor.matmul(out=psp1[:], lhsT=gpv_sb, rhs=mh_g[:, 0:64],
                                     start=True, stop=True)
                    p1 = st.tile([3, 64], f32)
                    nc.scalar.copy(out=p1[:], in_=psp1[:])
                    psp1t = stp.tile([64, 3], f32, tag="stsc", space="PSUM")
                    nc.tensor.matmul(out=psp1t[:], lhsT=p1[:], rhs=ident[0:3, 0:3],
                                     is_transpose=True, start=True, stop=True)
                    p1t = st.tile([64, 3], f32)
                    nc.scalar.copy(out=p1t[:], in_=psp1t[:])
                    psm3 = stp.tile([3, 3], f32, tag="stsc", space="PSUM")
                    nc.tensor.matmul(out=psm3[:], lhsT=p1t[:], rhs=gpv_sb,
                                     start=True, stop=True)
                    m3 = st.tile([3, 3], f32)
                    nc.scalar.mul(out=m3[:], in_=psm3[:], mul=1.0 / ntot)
                    psmu3 = stp.tile([3, 1], f32, tag="stsc", space="PSUM")
                    nc.tensor.matmul(out=psmu3[:], lhsT=gpv_sb, rhs=mud[:],
                                     start=True, stop=True)
                    mu3 = st.tile([3, 1], f32)
                    nc.scalar.copy(out=mu3[:], in_=psmu3[:])
                    psm3r = stp.tile([1, 3], f32, tag="stsc", space="PSUM")
                    nc.tensor.matmul(out=psm3r[:], lhsT=mu3[:], rhs=ident[0:3, 0:3],
                                     is_transpose=True, start=True, stop=True)
                    mu3r = st.tile([1, 3], f32)
                    nc.scalar.copy(out=mu3r[:], in_=psm3r[:])
                    pso3 = stp.tile([3, 3], f32, tag="stsc", space="PSUM")
                    nc.tensor.matmul(out=pso3[:], lhsT=mu3r[:], rhs=mu3r[:],
                                     start=True, stop=True)
                    nc.vector.tensor_sub(out=m3[:], in0=m3[:], in1=pso3[:])
                    wwt = w1ww_sb[:, 64:80]
                    psq1 = stp.tile([3, K], f32, tag="stsc", space="PSUM")
                    nc.tensor.matmul(out=psq1[:], lhsT=m3[:], rhs=wwt,
                                     start=True, stop=True)
                    prod = st.tile([3, K], f32)
                    nc.vector.tensor_mul(out=prod[:], in0=psq1[:], in1=wwt)
                    ones3b = st.tile([3, 1], f32, tag="ones3b")
                    nc.vector.memset(ones3b[:], 1.0)
                    psv3 = stp.tile([K, 1], f32, tag="stsc", space="PSUM")
                    nc.tensor.matmul(out=psv3[:], lhsT=prod[:], rhs=ones3b[:],
                                     start=True, stop=True)
                    s3 = st.tile([K, 1], f32)
                    v3sb = st.tile([K, 1], f32, tag="v3sb")
                    nc.vector.tensor_scalar_add(v3sb[:], psv3[:], EPS)
                    nc.scalar.activation(out=s3[:], in_=v3sb[:], func=AF.Sqrt)
                    nc.vector.reciprocal(out=s3[:], in_=s3[:])
                    nc.vector.tensor_mul(out=s3[:], in0=s3[:], in1=gwc)
                    psw3 = stp.tile([K, 1], f32, tag="stsc", space="PSUM")
                    nc.tensor.matmul(out=psw3[:], lhsT=wwt, rhs=mu3[:],
                                     start=True, stop=True)
                    inv3 = st.tile([K, 1], f32)
                    nc.vector.reciprocal(out=inv3[:], in_=s3[:])
                    nc.vector.tensor_mul(out=inv3[:], in0=inv3[:], in1=bewc)
                    cc3 = st.tile([K, 1], f32)
                    nc.vector.tensor_sub(out=cc3[:], in0=inv3[:], in1=psw3[:])
                    psr = stp.tile([1, K], f32, tag="stsc", space="PSUM")
                    s3r = st.tile([1, K], f32)
                    nc.tensor.matmul(out=psr[:], lhsT=s3[:], rhs=ident[0:K, 0:K],
                                     is_transpose=True, start=True, stop=True)
                    nc.scalar.copy(out=s3r[:], in_=psr[:])
                    psr2 = stp.tile([1, K], f32, tag="stsc", space="PSUM")
                    cc3r = st.tile([1, K], f32)
                    nc.tensor.matmul(out=psr2[:], lhsT=cc3[:], rhs=ident[0:K, 0:K],
                                     is_transpose=True, start=True, stop=True)
                    nc.scalar.copy(out=cc3r[:], in_=psr2[:])
                    s3rep = st.tile([128, K], f32)
                    nc.gpsimd.partition_broadcast(s3rep[:], s3r[:])
                    cc3rep = st.tile([128, K], f32)
                    nc.gpsimd.partition_broadcast(cc3rep[:], cc3r[:])
                    nc.vector.tensor_add(
                        out=wdp_all[:],
                        in0=wdiff_all[:],
                        in1=cc3rep[:].rearrange("p (o k) -> p o k", o=1).broadcast_to([128, nch, K]))
                    nc.scalar.activation(out=wdp_all[:], in_=wdp_all[:], func=AF.Relu)
                    nc.vector.tensor_mul(
                        out=wdp_all[:], in0=wdp_all[:],
                        in1=s3rep[:].rearrange("p (o k) -> p o k", o=1).broadcast_to([128, nch, K]))

                # ---------- r1 = relu(dh^T + c1) in SBUF ----------
                with tc.tile_pool(name="r1p", bufs=2, space="PSUM") as r1p:
                    for ci in range(nch):
                        for grp in range(4):
                            psdht = r1p.tile([64, 512], f32, tag="psdht", space="PSUM")
                            for k2 in range(4):
                                k = grp * 4 + k2
                                nc.tensor.matmul(
                                    out=psdht[:, k2 * 128:(k2 + 1) * 128],
                                    lhsT=dh_all[:, ci * K * 65 + k * 65:ci * K * 65 + k * 65 + 64],
                                    rhs=ident_bf[:], start=True, stop=True)
                            nc.scalar.activation(
                                out=r1_sb[:, ci * 2048 + grp * 512:ci * 2048 + (grp + 1) * 512],
                                in_=psdht[:], func=AF.Relu, bias=c1[:])

            # ---------- phase B2: BN2 stats (G gathered once, kept) ----------
            with tc.tile_pool(name="gp", bufs=1) as gp, \
                 tc.tile_pool(name="b2p", bufs=2, space="PSUM") as b2p:
                G_all = gp.tile([128, nch * K * C], bf)
                for ci in range(nch):
                    G2 = G_all[:, ci * K * C:(ci + 1) * K * C].rearrange(
                        "p (k c) -> p k c", k=K)
                    for k in range(K):
                        nc.gpsimd.indirect_dma_start(
                            out=G2[:, k, :], out_offset=None, in_=garr[:],
                            in_offset=bass.IndirectOffsetOnAxis(
                                ap=idx_all[:, ci * K + k:ci * K + k + 1], axis=0))
                    for grp in range(4):
                        psxt = b2p.tile([128, 512], f32, tag="psxt", space="PSUM")
                        nc.tensor.matmul(
                            out=psxt[:], lhsT=w2bt1_bf[:],
                            rhs=r1_sb[:, ci * 2048 + grp * 512:ci * 2048 + (grp + 1) * 512],
                            start=True, stop=False, skip_group_check=True)
                        for k2 in range(4):
                            k = grp * 4 + k2
                            nc.tensor.matmul(
                                out=psxt[:, k2 * 128:(k2 + 1) * 128],
                                lhsT=G2[:, k, :], rhs=ident_bf[:],
                                start=False, stop=(k2 == 3), skip_group_check=True)
                        nc.vector.bn_stats(
                            out=bn_all[:, (ci * 4 + grp) * 6:(ci * 4 + grp + 1) * 6],
                            in_=psxt[:])

                # ---------- AR2 + BN2 folding ----------
                with tc.tile_pool(name="s2t", bufs=1) as s2t, \
                     tc.tile_pool(name="s2p", bufs=2, space="PSUM") as s2p:
                    bnag = s2t.tile([128, 2], f32)
                    nc.vector.bn_aggr(out=bnag[:],
                                      in_=bn_all[:].rearrange("p (g s) -> p g s", s=6))
                    pay = s2t.tile([128, 2], f32)
                    nc.vector.tensor_copy(out=pay[:, 0:1], in_=bnag[:, 0:1])
                    msq = s2t.tile([128, 1], f32)
                    nc.scalar.activation(out=msq[:], in_=bnag[:, 0:1], func=AF.Square)
                    nc.vector.tensor_add(out=pay[:, 1:2], in0=bnag[:, 1:2], in1=msq[:])
                    nc.sync.dma_start(ar2i[:], pay[:])
                    nc.gpsimd.collective_compute(
                        "AllReduce", mybir.AluOpType.add,
                        ins=[ar2i[:]], outs=[ar2o[:]], replica_groups=rg)
                    arg = s2t.tile([128, 2], f32)
                    nc.sync.dma_start(arg[:], ar2o[:])
                    mux = s2t.tile([128, 1], f32)
                    nc.vector.tensor_scalar_mul(mux[:], arg[:, 0:1], 1.0 / n_cores)
                    ex2 = s2t.tile([128, 1], f32)
                    nc.vector.tensor_scalar_mul(ex2[:], arg[:, 1:2], 1.0 / n_cores)
                    mxs = s2t.tile([128, 1], f32)
                    nc.scalar.activation(out=mxs[:], in_=mux[:], func=AF.Square)
                    varx = s2t.tile([128, 1], f32)
                    nc.vector.tensor_sub(out=varx[:], in0=ex2[:], in1=mxs[:])
                    s2v = s2t.tile([128, 1], f32)
                    nc.vector.tensor_scalar_add(varx[:], varx[:], EPS)
                    nc.scalar.activation(out=s2v[:], in_=varx[:], func=AF.Sqrt)
                    nc.vector.reciprocal(out=s2v[:], in_=s2v[:])
                    nc.vector.tensor_mul(out=s2v[:], in0=s2v[:], in1=g2c)
                    c2p = s2t.tile([128, 1], f32)
                    nc.vector.tensor_mul(out=c2p[:], in0=mux[:], in1=s2v[:])
                    nc.vector.tensor_sub(out=c2p[:], in0=be2c, in1=c2p[:])
                    psr3 = s2p.tile([1, 128], f32, tag="s2sc", space="PSUM")
                    nc.tensor.matmul(out=psr3[:], lhsT=s2v[:], rhs=ident[:],
                                     is_transpose=True, start=True, stop=True)
                    s2row = s2t.tile([1, 128], f32)
                    nc.scalar.copy(out=s2row[:], in_=psr3[:])
                    psr4 = s2p.tile([1, 128], f32, tag="s2sc", space="PSUM")
                    nc.tensor.matmul(out=psr4[:], lhsT=c2p[:], rhs=ident[:],
                                     is_transpose=True, start=True, stop=True)
                    nc.scalar.copy(out=c2row[:], in_=psr4[:])
                    nc.gpsimd.partition_broadcast(s2rep[:], s2row[:])
                    s2rep64 = s2t.tile([64, C], f32)
                    nc.gpsimd.partition_broadcast(s2rep64[:], s2row[:])
                    nc.vector.tensor_mul(out=w2bt2[:], in0=w2bt1[:], in1=s2rep64[:])
                    nc.scalar.copy(out=w2bt2_bf[:], in_=w2bt2[:])
                    nc.scalar.copy(out=c2row_bf[:], in_=c2row[:])
                    nc.scalar.copy(out=s2rep_bf[:], in_=s2rep[:])

                # ---------- phase C ----------
                with tc.tile_pool(name="c1p", bufs=2) as cp, \
                     tc.tile_pool(name="cpp", bufs=2, space="PSUM") as cpp, \
                     tc.tile_pool(name="cop", bufs=3) as cop:
                    for ci in range(nch):
                        G3 = G_all[:, ci * K * C:(ci + 1) * K * C].rearrange(
                            "p (k c) -> p k c", k=K)
                        nc.vector.tensor_mul(
                            out=G3, in0=G3,
                            in1=s2rep_bf[:].rearrange("p (o c) -> p o c", o=1).broadcast_to(
                                [128, K, C]))
                        psot = cpp.tile([128, 128], f32, tag="psot", space="PSUM")
                        for grp in range(4):
                            psz = cpp.tile([128, 512], f32, tag="psz", space="PSUM")
                            nc.tensor.matmul(
                                out=psz[:], lhsT=ident_bf[:],
                                rhs=G3[:, grp * 4:(grp + 1) * 4, :].rearrange(
                                    "p k c -> p (k c)"),
                                start=True, stop=False, skip_group_check=True)
                            nc.tensor.matmul(
                                out=psz[:], lhsT=onesrow_bf[:],
                                rhs=c2row_bf[:].rearrange("o (d c) -> o d c", d=1).broadcast_to(
                                    [1, 4, C]),
                                start=False, stop=False, skip_group_check=True)
                            for k2 in range(4):
                                k = grp * 4 + k2
                                zsl = psz[:, k2 * 128:(k2 + 1) * 128]
                                nc.tensor.matmul(
                                    out=zsl,
                                    lhsT=r1_sb[:, ci * 2048 + k * 128:ci * 2048 + (k + 1) * 128],
                                    rhs=w2bt2_bf[:], start=False,
                                    stop=(k2 == 3),
                                    skip_group_check=True)
                                ek = cp.tile([128, 128], f32, tag="ek")
                                nc.scalar.activation(
                                    out=ek[:], in_=zsl, func=AF.Relu,
                                    scale=wdp_all[:, ci * K + k:ci * K + k + 1])
                                nc.tensor.matmul(out=psot[:], lhsT=ek[:], rhs=ident[:],
                                                 is_transpose=True, start=(k == 0),
                                                 stop=(k == K - 1), skip_group_check=True)
                        osb = cop.tile([128, 128], f16, tag="osb")
                        nc.scalar.copy(out=osb[:], in_=psot[:])
                        nc.sync.dma_start(y16[:, ci * 128:(ci + 1) * 128], osb[:])

    nc.finalize()
    return nc


_RUNNER = None
_POOL = ThreadPoolExecutor(8)


def _make_runner(nc, n_cores):
    import jax
    from jax.experimental.shard_map import shard_map
    from jax.sharding import Mesh, PartitionSpec
    from concourse import bass2jax, mybir as mb
    from concourse.bass2jax import partition_id_tensor

    partition_name = nc.partition_id_tensor.name if nc.partition_id_tensor else None
    in_names, out_names, out_avals = [], [], []
    for alloc in nc.m.functions[0].allocations:
        if not isinstance(alloc, mb.MemoryLocationSet):
            continue
        name = alloc.memorylocations[0].name
        if alloc.kind == "ExternalInput":
            if name != partition_name:
                in_names.append(name)
        elif alloc.kind == "ExternalOutput":
            shape = tuple(alloc.tensor_shape)
            dtype = mb.dt.np(alloc.dtype)
            out_names.append(name)
            out_avals.append(jax.core.ShapedArray(shape, dtype))
    n_params = len(in_names)
    all_in_names = list(in_names) + list(out_names)
    if partition_name is not None:
        all_in_names.append(partition_name)

    def _body(*args):
        operands = list(args)
        if partition_name is not None:
            operands.append(partition_id_tensor())
        outs = bass2jax._bass_exec_p.bind(
            *operands,
            out_avals=tuple(out_avals),
            in_names=tuple(all_in_names),
            out_names=tuple(out_names),
            lowering_input_output_aliases=(),
            sim_require_finite=True,
            sim_require_nnan=True,
            nc=nc,
        )
        return tuple(outs)

    import numpy as _np
    devices = jax.devices()[:n_cores]
    mesh = Mesh(_np.asarray(devices), ("core",))
    n_outs = len(out_names)
    sharded = jax.jit(
        shard_map(_body, mesh=mesh,
                  in_specs=(PartitionSpec("core"),) * (n_params + n_outs),
                  out_specs=(PartitionSpec("core"),) * n_outs,
                  check_rep=False),
        keep_unused=True)
    return dict(fn=sharded, in_names=in_names, out_names=out_names,
                out_avals=out_avals, mesh=mesh)


def _get_runner(rn):
    global _RUNNER
    if _RUNNER is not None:
        return _RUNNER
    import jax
    import jax.numpy as jnp
    from jax.sharding import NamedSharding, PartitionSpec
    from concourse.bass2jax import install_neuronx_cc_hook
    install_neuronx_cc_hook()
    r = _make_runner(build_full(rn), N_CORES)
    shd = NamedSharding(r["mesh"], PartitionSpec("core"))
    # dummy output buffers (kernel fully overwrites outputs; reused each call)
    dummies = []
    for av in r["out_avals"]:
        dummies.append(jnp.zeros((N_CORES * av.shape[0], *av.shape[1:]),
                                 av.dtype, device=shd))
    jax.block_until_ready(dummies)
    r["dummies"] = dummies
    _RUNNER = r
    return _RUNNER


def kernel(**inputs):
    F_E = np.asarray(inputs["F_E"], dtype=np.float32)
    Q = np.asarray(inputs["Q_prime"], dtype=np.float32)
    rn = F_E.shape[2]
    half = rn // 2
    r = _get_runner(rn)

    # int8 feature quantization, threaded per batch
    fe8_st = np.empty((B, 2, C, rn), np.int8)
    scale = np.empty((B, C), np.float32)

    def quant(b):
        amax = np.abs(F_E[b]).max(axis=1)
        s = np.maximum(amax, 1e-30) / 127.0
        scale[b] = s
        q = np.rint(F_E[b] * (1.0 / s)[:, None]).astype(np.int8)
        fe8_st[b, 0] = q
        fe8_st[b, 1] = q
    qfuts = [_POOL.submit(quant, b) for b in range(B)]

    q3c_st = np.ascontiguousarray(np.repeat(Q, 2, axis=0)).reshape(2 * B * 3, rn)
    q3q_st = np.ascontiguousarray(
        Q.reshape(B, 3, 2, half).transpose(0, 2, 1, 3)).reshape(2 * B * 3, half)
    W1 = np.asarray(inputs["W1"], np.float32)
    Ww = np.asarray(inputs["Ww"], np.float32)
    W2 = np.asarray(inputs["W2"], np.float32)
    wpk = np.zeros((B, 128, 284), np.float32)
    wpk[:, :, 0:128] = W2[:, :C].T
    wpk[:, :, 128:192] = W2[:, C:]
    wpk[:, :, 192] = np.asarray(inputs["g2"], np.float32)
    wpk[:, :, 193] = np.asarray(inputs["be2"], np.float32)
    wpk[:, 0:3, 196:276] = np.concatenate([W1.T, Ww.T], axis=1)
    wpk[:, 0:64, 276:279] = np.linalg.pinv(W1).T.astype(np.float32)
    wpk[:, 0:64, 279] = np.asarray(inputs["g1"], np.float32)
    wpk[:, 0:64, 280] = np.asarray(inputs["be1"], np.float32)
    wpk[:, 0:K, 281] = np.asarray(inputs["gw"], np.float32)
    wpk[:, 0:K, 282] = np.asarray(inputs["bew"], np.float32)
    for f in qfuts:
        f.result()
    wpk[:, :, 194] = scale
    wpk_st = np.repeat(wpk, 2, axis=0).reshape(N_CORES * 128, 284)

    args = dict(q3c=q3c_st, q3q=q3q_st, fe8=fe8_st.reshape(2 * B * C, rn),
                wpk=wpk_st)
    out = r["fn"](*[args[nm] for nm in r["in_names"]], *r["dummies"])
    y = np.asarray(out[r["out_names"].index("y16")])  # (2*B*C, half) f16

    res = np.empty((B, C, rn), np.float32)
    v = y.reshape(B, 2, C, half)

    def asm(b):
        np.add(F_E[b, :, :half], v[b, 0], out=res[b, :, :half])
        np.add(F_E[b, :, half:], v[b, 1], out=res[b, :, half:])
    list(_POOL.map(asm, range(B)))
    return res


# revision 15
# speedup vs baseline: 1.1336x; 1.1336x over previous
"""Trainium2 Bass kernel for nn_LocalRefinementUnit (KNN local refinement).

Sharding: 8 cores = (batch b = core//2) x (half h = core%2 of the 4096 points).
All candidate-side arrays (B5, recs, garr) are kept in GLOBAL point order, so
the two cores of a pair hold identical candidate state and only the query-side
inputs differ. This removes all host-side rolls.

Two pipelined device programs per call (the axon tunnel has ~80ms dispatch
latency but back-to-back calls pipeline, and H2D overlaps exec):

  call1 (q3 only, tiny upload):  B5/A5q, recs (h|dW records), per-chunk -d2
      via PE matmul, exact top-16, record gathers, delta-h, moment psum,
      AR1 -> BN1/BN3 stats, r1 = relu(dh^T + c1) -> DRAM, wdp weights.
  call2 (fe fp16 upload overlaps call1):  garr = W2a^T fe, B2 bn_stats from
      r1/garr gathers, AR2 -> BN2 fold, garr rescale, phase C -> weighted
      (fp16).  Residual add with full-precision F_E happens on host.
"""
import numpy as np
from concurrent.futures import ThreadPoolExecutor

import concourse.bass as bass
import concourse.mybir as mybir
import concourse.tile as tile
from concourse import bacc
from concourse.masks import make_identity

_POOL = ThreadPoolExecutor(8)

f32 = mybir.dt.float32
f16 = mybir.dt.float16
bf = mybir.dt.bfloat16
u32 = mybir.dt.uint32
AF = mybir.ActivationFunctionType

B, C, K = 4, 128, 16
EPS = 1e-5
N_CORES = 8
REC = 128          # record elems (f32): [h 64 | dW 16 | pad 48] = 512B


def build_knn(rn=4096, n_cores=N_CORES):
    half = rn // 2
    nch = half // 128           # query chunks of 128
    nsc = rn // 128             # candidate chunks of 128 points
    ntot = n_cores * half * K   # global BN row count

    nc = bacc.Bacc("TRN2", target_bir_lowering=False, debug=False,
                   num_devices=n_cores, enable_asserts=False)

    q3c = nc.dram_tensor("q3c", [3, rn], f32, kind="ExternalInput").ap()
    q3q = nc.dram_tensor("q3q", [3, half], f32, kind="ExternalInput").ap()
    # wp1[64,88]: [:,0:80] rows0-2 = [W1.T|Ww.T]; [:,80:83]=pinv(W1).T;
    # [:,83]=g1; [:,84]=be1; [:,85] rows0-16=gw; [:,86]=bew
    wp1 = nc.dram_tensor("wp1", [64, 88], f32, kind="ExternalInput").ap()

    idxo = nc.dram_tensor("idxo", [128, nch * K], u32, kind="ExternalOutput").ap()
    r1o = nc.dram_tensor("r1o", [64, nch * K * 128], bf, kind="ExternalOutput").ap()
    wdpo = nc.dram_tensor("wdpo", [128, nch * K], f32, kind="ExternalOutput").ap()
    s1o = nc.dram_tensor("s1o", [64, 1], f32, kind="ExternalOutput").ap()

    recs = nc.dram_tensor("recs", [rn, REC], f32).ap()
    ar1i = nc.dram_tensor("ar1i", [64, 65], f32).ap()
    ar1o = nc.dram_tensor("ar1o", [64, 65], f32, addr_space="Shared").ap()
    rg = [list(range(n_cores))]

    with tile.TileContext(nc) as tc:
        with tc.tile_pool(name="persist", bufs=1) as pp, \
             tc.tile_pool(name="ppsum", bufs=1, space="PSUM") as ppp:
            ident = pp.tile([128, 128], f32)
            make_identity(nc, ident[:])
            ones128 = pp.tile([128, 1], f32)
            nc.vector.memset(ones128[:], 1.0)

            wp1_sb = pp.tile([64, 88], f32)
            nc.sync.dma_start(wp1_sb[:], wp1[:])
            w1ww_sb = wp1_sb[0:3, 0:80]
            gpv_sb = wp1_sb[:, 80:83]
            g1c = wp1_sb[:, 83:84]
            be1c = wp1_sb[:, 84:85]
            gwc = wp1_sb[0:K, 85:86]
            bewc = wp1_sb[0:K, 86:87]

            # B5 = [q; 1; -sq] (candidates), A5q = [2q; -sq; 1] (queries)
            B5 = pp.tile([5, rn], f32)
            A5q = pp.tile([5, half], f32)
            q3q_sb = pp.tile([3, half], f32)
            nc.sync.dma_start(B5[0:3, :], q3c[:])
            nc.sync.dma_start(q3q_sb[:], q3q[:])

            dh_all = pp.tile([128, nch * K * 65], f32)
            idx_all = pp.tile([128, nch * K], u32)
            wdiff_all = pp.tile([128, nch * K], f32)
            wdp_all = pp.tile([128, nch * K], f32)
            mh_g = pp.tile([64, 65], f32)
            s1 = pp.tile([64, 1], f32)
            c1 = pp.tile([64, 1], f32)
            ps_mh = ppp.tile([64, 65], f32, space="PSUM")
            nc.vector.memset(
                dh_all[:].rearrange("p (g o) -> p g o", o=65)[:, :, 64:65], 1.0)

            # ---------- setup: squared norms + h|dW records ----------
            with tc.tile_pool(name="su", bufs=1) as su, \
                 tc.tile_pool(name="su2", bufs=2) as su2, \
                 tc.tile_pool(name="sup", bufs=2, space="PSUM") as sup:
                ones3 = su.tile([3, 1], f32, tag="ones3")
                nc.vector.memset(ones3[:], 1.0)
                onesr = su.tile([1, rn], f32, tag="onesr")
                nc.vector.memset(onesr[:], 1.0)
                nsqr = su.tile([1, rn], f32, tag="nsqr")
                q3sq = su.tile([3, rn], f32, tag="q3sq")
                nc.scalar.activation(out=q3sq[:], in_=B5[0:3, :], func=AF.Square)
                for i in range(rn // 512):
                    pssq = sup.tile([1, 512], f32, tag="pssq", space="PSUM")
                    nc.tensor.matmul(out=pssq[:], lhsT=ones3[:],
                                     rhs=q3sq[:, i * 512:(i + 1) * 512],
                                     start=True, stop=True)
                    nc.scalar.mul(out=nsqr[:, i * 512:(i + 1) * 512], in_=pssq[:],
                                  mul=-1.0)
                nc.sync.dma_start(B5[3:4, :], onesr[:])
                nc.sync.dma_start(B5[4:5, :], nsqr[:])
                # query side
                nsqq = su.tile([1, half], f32, tag="nsqq")
                qqsq = su.tile([3, half], f32, tag="qqsq")
                nc.scalar.activation(out=qqsq[:], in_=q3q_sb[:], func=AF.Square)
                for i in range(half // 512):
                    psq = sup.tile([1, 512], f32, tag="pssq", space="PSUM")
                    nc.tensor.matmul(out=psq[:], lhsT=ones3[:],
                                     rhs=qqsq[:, i * 512:(i + 1) * 512],
                                     start=True, stop=True)
                    nc.scalar.mul(out=nsqq[:, i * 512:(i + 1) * 512], in_=psq[:],
                                  mul=-1.0)
                nc.scalar.mul(out=A5q[0:3, :], in_=q3q_sb[:], mul=2.0)
                nc.sync.dma_start(A5q[3:4, :], nsqq[:])
                nc.sync.dma_start(A5q[4:5, :], onesr[:, 0:half])
                for i in range(nsc):
                    sl = slice(i * 128, (i + 1) * 128)
                    psh = sup.tile([128, 80], f32, tag="psh", space="PSUM")
                    nc.tensor.matmul(out=psh[:], lhsT=B5[0:3, sl],
                                     rhs=w1ww_sb[:], start=True, stop=True)
                    hsb = su2.tile([128, 80], f32, tag="hsb")
                    nc.scalar.copy(out=hsb[:], in_=psh[:])
                    nc.sync.dma_start(recs[sl, 0:80], hsb[:])

            # ---------- phase A + B1 ----------
            with tc.tile_pool(name="a1", bufs=1) as a1, \
                 tc.tile_pool(name="a2", bufs=2) as a2, \
                 tc.tile_pool(name="ap2", bufs=2, space="PSUM") as ap2:
                for ci in range(nch):
                    qsl = slice(ci * 128, (ci + 1) * 128)
                    vals = a1.tile([128, rn], f32, tag="vals")
                    qw = min(1024, rn)
                    for qd in range(rn // qw):
                        psd = ap2.tile([128, qw], f32, tag="psd", space="PSUM")
                        for hh in range(qw // 512):
                            nc.tensor.matmul(
                                out=psd[:, hh * 512:(hh + 1) * 512], lhsT=A5q[:, qsl],
                                rhs=B5[:, qd * qw + hh * 512:qd * qw + (hh + 1) * 512],
                                start=True, stop=True)
                        nc.scalar.copy(out=vals[:, qd * qw:qd * qw + 512],
                                       in_=psd[:, 0:512])
                        if qw > 512:
                            nc.scalar.copy(out=vals[:, qd * qw + 512:(qd + 1) * qw],
                                           in_=psd[:, 512:1024])
                    nseg = 16
                    sv = a2.tile([128, nseg * 8], f32, tag="sv")
                    for sgi in range(nseg):
                        nc.vector.max(out=sv[:, sgi * 8:(sgi + 1) * 8],
                                      in_=vals[:, sgi * (rn // 16):(sgi + 1) * (rn // 16)])
                    m1 = a2.tile([128, 8], f32, tag="m1")
                    m2 = a2.tile([128, 8], f32, tag="m2")
                    sv2 = a2.tile([128, nseg * 8], f32, tag="sv2")
                    nc.vector.max(out=m1[:], in_=sv[:])
                    nc.vector.match_replace(out=sv2[:], in_to_replace=m1[:],
                                            in_values=sv[:], imm_value=-1e30)
                    nc.vector.max(out=m2[:], in_=sv2[:])
                    nc.vector.max_index(out=idx_all[:, ci * K:ci * K + 8],
                                        in_max=m1[:], in_values=vals[:])
                    nc.vector.max_index(out=idx_all[:, ci * K + 8:ci * K + 16],
                                        in_max=m2[:], in_values=vals[:])

                    # B1: gather records, delta-h, moments
                    G = a2.tile([128, K, REC], f32, tag="G")
                    for k in range(K):
                        nc.gpsimd.indirect_dma_start(
                            out=G[:, k, :], out_offset=None, in_=recs[:],
                            in_offset=bass.IndirectOffsetOnAxis(
                                ap=idx_all[:, ci * K + k:ci * K + k + 1], axis=0))
                    psh = ap2.tile([128, 80], f32, tag="psh2", space="PSUM")
                    nc.tensor.matmul(out=psh[:], lhsT=q3q_sb[:, qsl],
                                     rhs=w1ww_sb[:], start=True, stop=True)
                    hq = a2.tile([128, 80], f32, tag="hq")
                    nc.scalar.copy(out=hq[:], in_=psh[:])
                    dh_ci = dh_all[:, ci * K * 65:(ci + 1) * K * 65].rearrange(
                        "p (k j) -> p k j", k=K)[:, :, 0:64]
                    nc.vector.tensor_sub(out=dh_ci, in0=G[:, :, 0:64],
                                         in1=hq[:, 0:64].rearrange("p (o j) -> p o j", o=1).broadcast_to([128, K, 64]))
                    Gflat = G[:].rearrange("p k r -> p (k r)")
                    nc.vector.tensor_sub(out=wdiff_all[:, ci * K:(ci + 1) * K],
                                         in0=Gflat[:, 64:64 + 129 * (K - 1) + 1:129],
                                         in1=hq[:, 64:80])
                    for k in range(K):
                        base = ci * K * 65 + k * 65
                        dsl = dh_all[:, base:base + 64]
                        dsl65 = dh_all[:, base:base + 65]
                        st = (ci == 0 and k == 0)
                        sp = (ci == nch - 1 and k == K - 1)
                        nc.tensor.matmul(out=ps_mh[:], lhsT=dsl, rhs=dsl65,
                                         start=st, stop=sp, skip_group_check=True)

            # ---------- AR1 + BN1/BN3 stat folding + r1 ----------
            with tc.tile_pool(name="st", bufs=1) as st, \
                 tc.tile_pool(name="stp", bufs=2, space="PSUM") as stp:
                mh_sb = st.tile([64, 65], f32)
                nc.scalar.copy(out=mh_sb[:], in_=ps_mh[:])
                nc.sync.dma_start(ar1i[:], mh_sb[:])
                nc.gpsimd.collective_compute(
                    "AllReduce", mybir.AluOpType.add,
                    ins=[ar1i[:]], outs=[ar1o[:]], replica_groups=rg)
                nc.sync.dma_start(mh_g[:], ar1o[:])

                mud = st.tile([64, 1], f32)
                nc.vector.tensor_scalar_mul(mud[:], mh_g[:, 64:65], 1.0 / ntot)
                mask = st.tile([64, 64], f32)
                nc.vector.tensor_mul(out=mask[:], in0=mh_g[:, 0:64],
                                     in1=ident[0:64, 0:64])
                psd1 = stp.tile([64, 1], f32, tag="stsc", space="PSUM")
                nc.tensor.matmul(out=psd1[:], lhsT=mask[:], rhs=ones128[0:64, :],
                                 start=True, stop=True)
                var1 = st.tile([64, 1], f32)
                nc.scalar.mul(out=var1[:], in_=psd1[:], mul=1.0 / ntot)
                musq = st.tile([64, 1], f32)
                nc.scalar.activation(out=musq[:], in_=mud[:], func=AF.Square)
                nc.vector.tensor_sub(out=var1[:], in0=var1[:], in1=musq[:])
                rs1 = st.tile([64, 1], f32)
                nc.vector.tensor_scalar_add(var1[:], var1[:], EPS)
                nc.scalar.activation(out=rs1[:], in_=var1[:], func=AF.Sqrt)
                nc.vector.reciprocal(out=rs1[:], in_=rs1[:])
                nc.vector.tensor_mul(out=s1[:], in0=rs1[:], in1=g1c)
                inv1 = st.tile([64, 1], f32)
                nc.vector.reciprocal(out=inv1[:], in_=s1[:])
                nc.vector.tensor_mul(out=inv1[:], in0=inv1[:], in1=be1c)
                nc.vector.tensor_sub(out=c1[:], in0=inv1[:], in1=mud[:])

                # BN3 via pinv: M3 = G Mh G^T
                psp1 = stp.tile([3, 64], f32, tag="stsc", space="PSUM")
                nc.tensor.matmul(out=psp1[:], lhsT=gpv_sb, rhs=mh_g[:, 0:64],
                                 start=True, stop=True)
                p1 = st.tile([3, 64], f32)
                nc.scalar.copy(out=p1[:], in_=psp1[:])
                psp1t = stp.tile([64, 3], f32, tag="stsc", space="PSUM")
                nc.tensor.matmul(out=psp1t[:], lhsT=p1[:], rhs=ident[0:3, 0:3],
                                 is_transpose=True, start=True, stop=True)
                p1t = st.tile([64, 3], f32)
                nc.scalar.copy(out=p1t[:], in_=psp1t[:])
                psm3 = stp.tile([3, 3], f32, tag="stsc", space="PSUM")
                nc.tensor.matmul(out=psm3[:], lhsT=p1t[:], rhs=gpv_sb,
                                 start=True, stop=True)
                m3 = st.tile([3, 3], f32)
                nc.scalar.mul(out=m3[:], in_=psm3[:], mul=1.0 / ntot)
                psmu3 = stp.tile([3, 1], f32, tag="stsc", space="PSUM")
                nc.tensor.matmul(out=psmu3[:], lhsT=gpv_sb, rhs=mud[:],
                                 start=True, stop=True)
                mu3 = st.tile([3, 1], f32)
                nc.scalar.copy(out=mu3[:], in_=psmu3[:])
                psm3r = stp.tile([1, 3], f32, tag="stsc", space="PSUM")
                nc.tensor.matmul(out=psm3r[:], lhsT=mu3[:], rhs=ident[0:3, 0:3],
                                 is_transpose=True, start=True, stop=True)
                mu3r = st.tile([1, 3], f32)
                nc.scalar.copy(out=mu3r[:], in_=psm3r[:])
                pso3 = stp.tile([3, 3], f32, tag="stsc", space="PSUM")
                nc.tensor.matmul(out=pso3[:], lhsT=mu3r[:], rhs=mu3r[:],
                                 start=True, stop=True)
                nc.vector.tensor_sub(out=m3[:], in0=m3[:], in1=pso3[:])  # Cov3
                wwt = w1ww_sb[:, 64:80]
                psq1 = stp.tile([3, K], f32, tag="stsc", space="PSUM")
                nc.tensor.matmul(out=psq1[:], lhsT=m3[:], rhs=wwt,
                                 start=True, stop=True)
                prod = st.tile([3, K], f32)
                nc.vector.tensor_mul(out=prod[:], in0=psq1[:], in1=wwt)
                ones3b = st.tile([3, 1], f32, tag="ones3b")
                nc.vector.memset(ones3b[:], 1.0)
                psv3 = stp.tile([K, 1], f32, tag="stsc", space="PSUM")
                nc.tensor.matmul(out=psv3[:], lhsT=prod[:], rhs=ones3b[:],
                                 start=True, stop=True)
                s3 = st.tile([K, 1], f32)
                v3sb = st.tile([K, 1], f32, tag="v3sb")
                nc.vector.tensor_scalar_add(v3sb[:], psv3[:], EPS)
                nc.scalar.activation(out=s3[:], in_=v3sb[:], func=AF.Sqrt)
                nc.vector.reciprocal(out=s3[:], in_=s3[:])
                nc.vector.tensor_mul(out=s3[:], in0=s3[:], in1=gwc)
                psw3 = stp.tile([K, 1], f32, tag="stsc", space="PSUM")
                nc.tensor.matmul(out=psw3[:], lhsT=wwt, rhs=mu3[:],
                                 start=True, stop=True)
                inv3 = st.tile([K, 1], f32)
                nc.vector.reciprocal(out=inv3[:], in_=s3[:])
                nc.vector.tensor_mul(out=inv3[:], in0=inv3[:], in1=bewc)
                cc3 = st.tile([K, 1], f32)
                nc.vector.tensor_sub(out=cc3[:], in0=inv3[:], in1=psw3[:])
                psr = stp.tile([1, K], f32, tag="stsc", space="PSUM")
                s3r = st.tile([1, K], f32)
                nc.tensor.matmul(out=psr[:], lhsT=s3[:], rhs=ident[0:K, 0:K],
                                 is_transpose=True, start=True, stop=True)
                nc.scalar.copy(out=s3r[:], in_=psr[:])
                psr2 = stp.tile([1, K], f32, tag="stsc", space="PSUM")
                cc3r = st.tile([1, K], f32)
                nc.tensor.matmul(out=psr2[:], lhsT=cc3[:], rhs=ident[0:K, 0:K],
                                 is_transpose=True, start=True, stop=True)
                nc.scalar.copy(out=cc3r[:], in_=psr2[:])
                s3rep = st.tile([128, K], f32)
                nc.gpsimd.partition_broadcast(s3rep[:], s3r[:])
                cc3rep = st.tile([128, K], f32)
                nc.gpsimd.partition_broadcast(cc3rep[:], cc3r[:])
                nc.vector.tensor_add(
                    out=wdp_all[:],
                    in0=wdiff_all[:],
                    in1=cc3rep[:].rearrange("p (o k) -> p o k", o=1).broadcast_to([128, nch, K]))
                nc.scalar.activation(out=wdp_all[:], in_=wdp_all[:], func=AF.Relu)
                nc.vector.tensor_mul(
                    out=wdp_all[:], in0=wdp_all[:],
                    in1=s3rep[:].rearrange("p (o k) -> p o k", o=1).broadcast_to([128, nch, K]))
                nc.sync.dma_start(wdpo[:], wdp_all[:])
                nc.sync.dma_start(idxo[:], idx_all[:])
                nc.sync.dma_start(s1o[:], s1[:])

            # ---------- r1 = relu(dh^T + c1) -> DRAM ----------
            with tc.tile_pool(name="r1g", bufs=3) as r1g, \
                 tc.tile_pool(name="r1p", bufs=2, space="PSUM") as r1p:
                for ci in range(nch):
                    for grp in range(4):
                        psdht = r1p.tile([64, 512], f32, tag="psdht", space="PSUM")
                        for k2 in range(4):
                            k = grp * 4 + k2
                            nc.tensor.matmul(
                                out=psdht[:, k2 * 128:(k2 + 1) * 128],
                                lhsT=dh_all[:, ci * K * 65 + k * 65:ci * K * 65 + k * 65 + 64],
                                rhs=ident[:], is_transpose=True, start=True, stop=True)
                        r1t = r1g.tile([64, 512], bf, tag="r1t")
                        nc.scalar.activation(out=r1t[:], in_=psdht[:],
                                             func=AF.Relu, bias=c1[:])
                        nc.sync.dma_start(
                            r1o[:, ci * 2048 + grp * 512:ci * 2048 + (grp + 1) * 512],
                            r1t[:])

    nc.finalize()
    return nc


def build_main(rn=4096, n_cores=N_CORES):
    half = rn // 2
    nch = half // 128
    nsc = rn // 128

    nc = bacc.Bacc("TRN2", target_bir_lowering=False, debug=False,
                   num_devices=n_cores, enable_asserts=False)

    fe8 = nc.dram_tensor("fe8", [C, rn], mybir.dt.int8, kind="ExternalInput").ap()
    # wp2[128,196]: [:,0:128]=W2a.T; [:,128:192]=W2b.T transposed ([C,64]);
    # [:,192]=g2; [:,193]=be2; [:,194]=int8 dequant scale per channel
    wp2 = nc.dram_tensor("wp2", [128, 196], f32, kind="ExternalInput").ap()
    idxi = nc.dram_tensor("idxi", [128, nch * K], u32, kind="ExternalInput").ap()
    r1i = nc.dram_tensor("r1i", [64, nch * K * 128], bf, kind="ExternalInput").ap()
    wdpi = nc.dram_tensor("wdpi", [128, nch * K], f32, kind="ExternalInput").ap()
    s1i = nc.dram_tensor("s1i", [64, 1], f32, kind="ExternalInput").ap()

    y16 = nc.dram_tensor("y16", [C, half], f16, kind="ExternalOutput").ap()

    garr = nc.dram_tensor("garr", [rn, C], bf).ap()
    ar2i = nc.dram_tensor("ar2i", [C, 2], f32).ap()
    ar2o = nc.dram_tensor("ar2o", [C, 2], f32, addr_space="Shared").ap()
    rg = [list(range(n_cores))]

    with tile.TileContext(nc) as tc:
        with tc.tile_pool(name="persist", bufs=1) as pp, \
             tc.tile_pool(name="ppsum", bufs=1, space="PSUM") as ppp:
            ident = pp.tile([128, 128], f32)
            make_identity(nc, ident[:])
            ident_bf = pp.tile([128, 128], bf)
            nc.vector.tensor_copy(out=ident_bf[:], in_=ident[:])
            onesrow_bf = pp.tile([1, 128], bf)
            nc.vector.memset(onesrow_bf[:], 1.0)

            fe8_sb = pp.tile([C, rn], mybir.dt.int8)
            nc.sync.dma_start(fe8_sb[:], fe8[:])
            fe_sb = pp.tile([C, rn], f16)
            nc.vector.tensor_copy(out=fe_sb[:], in_=fe8_sb[:])
            wp2_sb = pp.tile([128, 196], f32)
            nc.sync.dma_start(wp2_sb[:], wp2[:])
            idx_sb = pp.tile([128, nch * K], u32)
            nc.sync.dma_start(idx_sb[:], idxi[:])
            wdp_sb = pp.tile([128, nch * K], f32)
            nc.sync.dma_start(wdp_sb[:], wdpi[:])
            s1 = pp.tile([64, 1], f32)
            nc.sync.dma_start(s1[:], s1i[:])
            r1_sb = pp.tile([64, nch * K * 128], bf)
            nc.sync.dma_start(r1_sb[:], r1i[:])

            # fold int8 dequant scale into W2a rows
            w2at_f = pp.tile([128, 128], f32)
            nc.vector.tensor_mul(out=w2at_f[:], in0=wp2_sb[:, 0:128],
                                 in1=wp2_sb[:, 194:195].broadcast_to([128, 128]))
            w2at16 = pp.tile([128, 128], f16)
            nc.scalar.copy(out=w2at16[:], in_=w2at_f[:])
            g2c = wp2_sb[:, 192:193]
            be2c = wp2_sb[:, 193:194]

            G_all = pp.tile([128, nch * K * C], bf)
            w2bt = pp.tile([64, C], f32)
            w2bt1 = pp.tile([64, C], f32)
            w2bt1_bf = pp.tile([64, C], bf)
            w2bt2 = pp.tile([64, C], f32)
            w2bt2_bf = pp.tile([64, C], bf)
            c2row = pp.tile([1, C], f32)
            c2row_bf = pp.tile([1, C], bf)
            s2rep = pp.tile([C, C], f32)
            s2rep_bf = pp.tile([C, C], bf)
            bn_all = pp.tile([128, nch * 4 * 6], f32)

            pswt = ppp.tile([64, 128], f32, space="PSUM")
            nc.tensor.matmul(out=pswt[:], lhsT=wp2_sb[:, 128:192], rhs=ident[:],
                             is_transpose=True, start=True, stop=True)
            nc.scalar.copy(out=w2bt[:], in_=pswt[:])
            nc.vector.tensor_mul(out=w2bt1[:], in0=w2bt[:],
                                 in1=s1[:].broadcast_to([64, C]))
            nc.scalar.copy(out=w2bt1_bf[:], in_=w2bt1[:])

            # ---------- garr = W2a^T fe ----------
            with tc.tile_pool(name="su2", bufs=2) as su2, \
                 tc.tile_pool(name="sup", bufs=2, space="PSUM") as sup:
                for i in range(nsc):
                    sl = slice(i * 128, (i + 1) * 128)
                    psg = sup.tile([128, C], f32, tag="psg", space="PSUM")
                    nc.tensor.matmul(out=psg[:], lhsT=fe_sb[:, sl],
                                     rhs=w2at16[:], start=True, stop=True)
                    gsb = su2.tile([128, C], bf, tag="gsb")
                    nc.scalar.copy(out=gsb[:], in_=psg[:])
                    nc.sync.dma_start(garr[sl, :], gsb[:])

            # ---------- phase B2: BN2 stats ----------
            with tc.tile_pool(name="b2p", bufs=2, space="PSUM") as b2p:
                for ci in range(nch):
                    G2 = G_all[:, ci * K * C:(ci + 1) * K * C].rearrange(
                        "p (k c) -> p k c", k=K)
                    for k in range(K):
                        nc.gpsimd.indirect_dma_start(
                            out=G2[:, k, :], out_offset=None, in_=garr[:],
                            in_offset=bass.IndirectOffsetOnAxis(
                                ap=idx_sb[:, ci * K + k:ci * K + k + 1], axis=0))
                    for grp in range(4):
                        psxt = b2p.tile([128, 512], f32, tag="psxt", space="PSUM")
                        nc.tensor.matmul(
                            out=psxt[:], lhsT=w2bt1_bf[:],
                            rhs=r1_sb[:, ci * 2048 + grp * 512:ci * 2048 + (grp + 1) * 512],
                            start=True, stop=False, skip_group_check=True)
                        for k2 in range(4):
                            k = grp * 4 + k2
                            nc.tensor.matmul(
                                out=psxt[:, k2 * 128:(k2 + 1) * 128],
                                lhsT=G2[:, k, :], rhs=ident_bf[:],
                                start=False, stop=(k2 == 3), skip_group_check=True)
                        nc.vector.bn_stats(
                            out=bn_all[:, (ci * 4 + grp) * 6:(ci * 4 + grp + 1) * 6],
                            in_=psxt[:])

            # ---------- AR2 + BN2 folding + garr rescale ----------
            with tc.tile_pool(name="s2t", bufs=1) as s2t, \
                 tc.tile_pool(name="s2p", bufs=2, space="PSUM") as s2p:
                bnag = s2t.tile([128, 2], f32)
                nc.vector.bn_aggr(out=bnag[:],
                                  in_=bn_all[:].rearrange("p (g s) -> p g s", s=6))
                pay = s2t.tile([128, 2], f32)
                nc.vector.tensor_copy(out=pay[:, 0:1], in_=bnag[:, 0:1])
                msq = s2t.tile([128, 1], f32)
                nc.scalar.activation(out=msq[:], in_=bnag[:, 0:1], func=AF.Square)
                nc.vector.tensor_add(out=pay[:, 1:2], in0=bnag[:, 1:2], in1=msq[:])
                nc.sync.dma_start(ar2i[:], pay[:])
                nc.gpsimd.collective_compute(
                    "AllReduce", mybir.AluOpType.add,
                    ins=[ar2i[:]], outs=[ar2o[:]], replica_groups=rg)
                arg = s2t.tile([128, 2], f32)
                nc.sync.dma_start(arg[:], ar2o[:])
                mux = s2t.tile([128, 1], f32)
                nc.vector.tensor_scalar_mul(mux[:], arg[:, 0:1], 1.0 / n_cores)
                ex2 = s2t.tile([128, 1], f32)
                nc.vector.tensor_scalar_mul(ex2[:], arg[:, 1:2], 1.0 / n_cores)
                mxs = s2t.tile([128, 1], f32)
                nc.scalar.activation(out=mxs[:], in_=mux[:], func=AF.Square)
                varx = s2t.tile([128, 1], f32)
                nc.vector.tensor_sub(out=varx[:], in0=ex2[:], in1=mxs[:])
                s2v = s2t.tile([128, 1], f32)
                nc.vector.tensor_scalar_add(varx[:], varx[:], EPS)
                nc.scalar.activation(out=s2v[:], in_=varx[:], func=AF.Sqrt)
                nc.vector.reciprocal(out=s2v[:], in_=s2v[:])
                nc.vector.tensor_mul(out=s2v[:], in0=s2v[:], in1=g2c)
                c2p = s2t.tile([128, 1], f32)
                nc.vector.tensor_mul(out=c2p[:], in0=mux[:], in1=s2v[:])
                nc.vector.tensor_sub(out=c2p[:], in0=be2c, in1=c2p[:])
                psr3 = s2p.tile([1, 128], f32, tag="s2sc", space="PSUM")
                nc.tensor.matmul(out=psr3[:], lhsT=s2v[:], rhs=ident[:],
                                 is_transpose=True, start=True, stop=True)
                s2row = s2t.tile([1, 128], f32)
                nc.scalar.copy(out=s2row[:], in_=psr3[:])
                psr4 = s2p.tile([1, 128], f32, tag="s2sc", space="PSUM")
                nc.tensor.matmul(out=psr4[:], lhsT=c2p[:], rhs=ident[:],
                                 is_transpose=True, start=True, stop=True)
                nc.scalar.copy(out=c2row[:], in_=psr4[:])
                nc.gpsimd.partition_broadcast(s2rep[:], s2row[:])
                s2rep64 = s2t.tile([64, C], f32)
                nc.gpsimd.partition_broadcast(s2rep64[:], s2row[:])
                nc.vector.tensor_mul(out=w2bt2[:], in0=w2bt1[:], in1=s2rep64[:])
                nc.scalar.copy(out=w2bt2_bf[:], in_=w2bt2[:])
                nc.scalar.copy(out=c2row_bf[:], in_=c2row[:])
                nc.scalar.copy(out=s2rep_bf[:], in_=s2rep[:])

            # ---------- phase C ----------
            with tc.tile_pool(name="c1p", bufs=2) as cp, \
                 tc.tile_pool(name="cpp", bufs=2, space="PSUM") as cpp, \
                 tc.tile_pool(name="cop", bufs=3) as cop:
                for ci in range(nch):
                    G3 = G_all[:, ci * K * C:(ci + 1) * K * C].rearrange(
                        "p (k c) -> p k c", k=K)
                    nc.vector.tensor_mul(
                        out=G3, in0=G3,
                        in1=s2rep_bf[:].rearrange("p (o c) -> p o c", o=1).broadcast_to(
                            [128, K, C]))
                    psot = cpp.tile([128, 128], f32, tag="psot", space="PSUM")
                    for grp in range(4):
                        psz = cpp.tile([128, 512], f32, tag="psz", space="PSUM")
                        nc.tensor.matmul(
                            out=psz[:], lhsT=ident_bf[:],
                            rhs=G3[:, grp * 4:(grp + 1) * 4, :].rearrange(
                                "p k c -> p (k c)"),
                            start=True, stop=False, skip_group_check=True)
                        nc.tensor.matmul(
                            out=psz[:], lhsT=onesrow_bf[:],
                            rhs=c2row_bf[:].rearrange("o (d c) -> o d c", d=1).broadcast_to(
                                [1, 4, C]),
                            start=False, stop=False, skip_group_check=True)
                        for k2 in range(4):
                            k = grp * 4 + k2
                            zsl = psz[:, k2 * 128:(k2 + 1) * 128]
                            nc.tensor.matmul(
                                out=zsl,
                                lhsT=r1_sb[:, ci * 2048 + k * 128:ci * 2048 + (k + 1) * 128],
                                rhs=w2bt2_bf[:], start=False,
                                stop=(k2 == 3),
                                skip_group_check=True)
                            ek = cp.tile([128, 128], f32, tag="ek")
                            nc.scalar.activation(
                                out=ek[:], in_=zsl, func=AF.Relu,
                                scale=wdp_sb[:, ci * K + k:ci * K + k + 1])
                            nc.tensor.matmul(out=psot[:], lhsT=ek[:], rhs=ident[:],
                                             is_transpose=True, start=(k == 0),
                                             stop=(k == K - 1), skip_group_check=True)
                    osb = cop.tile([128, 128], f16, tag="osb")
                    nc.scalar.copy(out=osb[:], in_=psot[:])
                    nc.sync.dma_start(y16[:, ci * 128:(ci + 1) * 128], osb[:])

    nc.finalize()
    return nc


_RUNNERS = None


def _make_runner(nc, n_cores):
    import jax
    from jax.experimental.shard_map import shard_map
    from jax.sharding import Mesh, PartitionSpec
    from concourse import bass2jax, mybir as mb
    from concourse.bass2jax import partition_id_tensor

    partition_name = nc.partition_id_tensor.name if nc.partition_id_tensor else None
    in_names, out_names, out_avals = [], [], []
    for alloc in nc.m.functions[0].allocations:
        if not isinstance(alloc, mb.MemoryLocationSet):
            continue
        name = alloc.memorylocations[0].name
        if alloc.kind == "ExternalInput":
            if name != partition_name:
                in_names.append(name)
        elif alloc.kind == "ExternalOutput":
            shape = tuple(alloc.tensor_shape)
            dtype = mb.dt.np(alloc.dtype)
            out_names.append(name)
            out_avals.append(jax.core.ShapedArray(shape, dtype))
    n_params = len(in_names)
    all_in_names = list(in_names) + list(out_names)
    if partition_name is not None:
        all_in_names.append(partition_name)

    def _body(*args):
        operands = list(args)
        if partition_name is not None:
            operands.append(partition_id_tensor())
        outs = bass2jax._bass_exec_p.bind(
            *operands,
            out_avals=tuple(out_avals),
            in_names=tuple(all_in_names),
            out_names=tuple(out_names),
            lowering_input_output_aliases=(),
            sim_require_finite=True,
            sim_require_nnan=True,
            nc=nc,
        )
        return tuple(outs)

    import numpy as _np
    devices = jax.devices()[:n_cores]
    mesh = Mesh(_np.asarray(devices), ("core",))
    n_outs = len(out_names)
    sharded = jax.jit(
        shard_map(_body, mesh=mesh,
                  in_specs=(PartitionSpec("core"),) * (n_params + n_outs),
                  out_specs=(PartitionSpec("core"),) * n_outs,
                  check_rep=False),
        keep_unused=True)
    return dict(fn=sharded, in_names=in_names, out_names=out_names,
                out_avals=out_avals, mesh=mesh)


def _get_runners(rn):
    global _RUNNERS
    if _RUNNERS is not None:
        return _RUNNERS
    import jax
    import jax.numpy as jnp
    from jax.sharding import NamedSharding, PartitionSpec
    from concourse.bass2jax import install_neuronx_cc_hook
    install_neuronx_cc_hook()
    r1 = _make_runner(build_knn(rn), N_CORES)
    r2 = _make_runner(build_main(rn), N_CORES)
    shd = NamedSharding(r1["mesh"], PartitionSpec("core"))
    # dummy output buffers (kernels fully overwrite outputs; reused each call)
    for r in (r1, r2):
        dummies = []
        for av in r["out_avals"]:
            dummies.append(jnp.zeros((N_CORES * av.shape[0], *av.shape[1:]),
                                     av.dtype, device=shd))
        jax.block_until_ready(dummies)
        r["dummies"] = dummies
    _RUNNERS = (r1, r2)
    return _RUNNERS


def kernel(**inputs):
    F_E = np.asarray(inputs["F_E"], dtype=np.float32)
    Q = np.asarray(inputs["Q_prime"], dtype=np.float32)
    rn = F_E.shape[2]
    half = rn // 2
    r1, r2 = _get_runners(rn)

    # int8 feature quantization in background threads (overlaps call1 dispatch)
    fe8_st = np.empty((B, 2, C, rn), np.int8)
    scale = np.empty((B, C), np.float32)

    def quant(b):
        amax = np.abs(F_E[b]).max(axis=1)
        s = np.maximum(amax, 1e-30) / 127.0
        scale[b] = s
        q = np.rint(F_E[b] * (1.0 / s)[:, None]).astype(np.int8)
        fe8_st[b, 0] = q
        fe8_st[b, 1] = q
    qfuts = [_POOL.submit(quant, b) for b in range(B)]

    # --- small host prep (query side + packed weights) ---
    q3c_st = np.ascontiguousarray(np.repeat(Q, 2, axis=0)).reshape(2 * B * 3, rn)
    q3q_st = np.ascontiguousarray(
        Q.reshape(B, 3, 2, half).transpose(0, 2, 1, 3)).reshape(2 * B * 3, half)
    W1 = np.asarray(inputs["W1"], np.float32)
    Ww = np.asarray(inputs["Ww"], np.float32)
    wp1 = np.zeros((64, 88), np.float32)
    wp1[0:3, 0:80] = np.concatenate([W1.T, Ww.T], axis=1)
    wp1[:, 80:83] = np.linalg.pinv(W1).T.astype(np.float32)
    wp1[:, 83] = np.asarray(inputs["g1"], np.float32)
    wp1[:, 84] = np.asarray(inputs["be1"], np.float32)
    wp1[0:K, 85] = np.asarray(inputs["gw"], np.float32)
    wp1[0:K, 86] = np.asarray(inputs["bew"], np.float32)
    wp1_st = np.broadcast_to(wp1, (N_CORES, 64, 88)).reshape(N_CORES * 64, 88)

    args1 = dict(q3c=q3c_st, q3q=q3q_st, wp1=wp1_st)
    out1 = r1["fn"](*[args1[nm] for nm in r1["in_names"]], *r1["dummies"])
    state = dict(zip(r1["out_names"], out1))

    # --- big feature upload dispatched while call1 is in flight ---
    W2 = np.asarray(inputs["W2"], np.float32)
    wp2 = np.zeros((B, 128, 196), np.float32)
    wp2[:, :, 0:128] = W2[:, :C].T
    wp2[:, :, 128:192] = W2[:, C:]       # (C,64) = (W2b^T)^T
    wp2[:, :, 192] = np.asarray(inputs["g2"], np.float32)
    wp2[:, :, 193] = np.asarray(inputs["be2"], np.float32)
    for f in qfuts:
        f.result()
    wp2[:, :, 194] = scale
    wp2_st = np.repeat(wp2, 2, axis=0).reshape(N_CORES * 128, 196)

    args2 = dict(fe8=fe8_st.reshape(2 * B * C, rn), wp2=wp2_st,
                 idxi=state["idxo"], r1i=state["r1o"], wdpi=state["wdpo"],
                 s1i=state["s1o"])
    out2 = r2["fn"](*[args2[nm] for nm in r2["in_names"]], *r2["dummies"])
    y = np.asarray(out2[r2["out_names"].index("y16")])  # (2*B*C, half) f16

    res = np.empty((B, C, rn), np.float32)
    v = y.reshape(B, 2, C, half)

    def asm(b):
        np.add(F_E[b, :, :half], v[b, 0], out=res[b, :, :half])
        np.add(F_E[b, :, half:], v[b, 1], out=res[b, :, half:])
    list(_POOL.map(asm, range(B)))
    return res


# revision 19
# speedup vs baseline: 1.3740x; 1.2120x over previous
"""Trainium2 Bass kernel for nn_LocalRefinementUnit (KNN local refinement).

Sharding: 8 cores = (batch b = core//2) x (half h = core%2 of the 4096 points).
All candidate-side arrays (B5, recs, garr) are kept in GLOBAL point order, so
the two cores of a pair hold identical candidate state and only the query-side
inputs differ. This removes all host-side rolls.

Two pipelined device programs per call (the axon tunnel has ~80ms dispatch
latency but back-to-back calls pipeline, and H2D overlaps exec):

  call1 (q3 only, tiny upload):  B5/A5q, recs (h|dW records), per-chunk -d2
      via PE matmul, exact top-16, record gathers, delta-h, moment psum,
      AR1 -> BN1/BN3 stats, r1 = relu(dh^T + c1) -> DRAM, wdp weights.
  call2 (fe fp16 upload overlaps call1):  garr = W2a^T fe, B2 bn_stats from
      r1/garr gathers, AR2 -> BN2 fold, garr rescale, phase C -> weighted
      (fp16).  Residual add with full-precision F_E happens on host.
"""
import numpy as np
from concurrent.futures import ThreadPoolExecutor

import concourse.bass as bass
import concourse.mybir as mybir
import concourse.tile as tile
from concourse import bacc
from concourse.masks import make_identity

_POOL = ThreadPoolExecutor(8)

f32 = mybir.dt.float32
f16 = mybir.dt.float16
bf = mybir.dt.bfloat16
u32 = mybir.dt.uint32
AF = mybir.ActivationFunctionType

B, C, K = 4, 128, 16
EPS = 1e-5
N_CORES = 8
REC = 128          # record elems (f32): [h 64 | dW 16 | pad 48] = 512B


def build_knn(rn=4096, n_cores=N_CORES):
    half = rn // 2
    nch = half // 128           # query chunks of 128
    nsc = rn // 128             # candidate chunks of 128 points
    ntot = n_cores * half * K   # global BN row count

    nc = bacc.Bacc("TRN2", target_bir_lowering=False, debug=False,
                   num_devices=n_cores, enable_asserts=False)

    q3c = nc.dram_tensor("q3c", [3, rn], f32, kind="ExternalInput").ap()
    q3q = nc.dram_tensor("q3q", [3, half], f32, kind="ExternalInput").ap()
    # wp1[64,88]: [:,0:80] rows0-2 = [W1.T|Ww.T]; [:,80:83]=pinv(W1).T;
    # [:,83]=g1; [:,84]=be1; [:,85] rows0-16=gw; [:,86]=bew
    wp1 = nc.dram_tensor("wp1", [64, 88], f32, kind="ExternalInput").ap()

    idxo = nc.dram_tensor("idxo", [128, nch * K], u32, kind="ExternalOutput").ap()
    r1o = nc.dram_tensor("r1o", [64, nch * K * 128], bf, kind="ExternalOutput").ap()
    wdpo = nc.dram_tensor("wdpo", [128, nch * K], f32, kind="ExternalOutput").ap()
    s1o = nc.dram_tensor("s1o", [64, 1], f32, kind="ExternalOutput").ap()

    recs = nc.dram_tensor("recs", [rn, REC], f32).ap()
    ar1i = nc.dram_tensor("ar1i", [64, 65], f32).ap()
    ar1o = nc.dram_tensor("ar1o", [64, 65], f32, addr_space="Shared").ap()
    rg = [list(range(n_cores))]

    with tile.TileContext(nc) as tc:
        with tc.tile_pool(name="persist", bufs=1) as pp, \
             tc.tile_pool(name="ppsum", bufs=1, space="PSUM") as ppp:
            ident = pp.tile([128, 128], f32)
            make_identity(nc, ident[:])
            ones128 = pp.tile([128, 1], f32)
            nc.vector.memset(ones128[:], 1.0)

            wp1_sb = pp.tile([64, 88], f32)
            nc.sync.dma_start(wp1_sb[:], wp1[:])
            w1ww_sb = wp1_sb[0:3, 0:80]
            gpv_sb = wp1_sb[:, 80:83]
            g1c = wp1_sb[:, 83:84]
            be1c = wp1_sb[:, 84:85]
            gwc = wp1_sb[0:K, 85:86]
            bewc = wp1_sb[0:K, 86:87]

            # B5 = [q; 1; -sq] (candidates), A5q = [2q; -sq; 1] (queries)
            B5 = pp.tile([5, rn], f32)
            A5q = pp.tile([5, half], f32)
            q3q_sb = pp.tile([3, half], f32)
            nc.sync.dma_start(B5[0:3, :], q3c[:])
            nc.sync.dma_start(q3q_sb[:], q3q[:])

            dh_all = pp.tile([128, nch * K * 65], f32)
            idx_all = pp.tile([128, nch * K], u32)
            wdiff_all = pp.tile([128, nch * K], f32)
            wdp_all = pp.tile([128, nch * K], f32)
            mh_g = pp.tile([64, 65], f32)
            s1 = pp.tile([64, 1], f32)
            c1 = pp.tile([64, 1], f32)
            ps_mh = ppp.tile([64, 65], f32, space="PSUM")
            nc.vector.memset(
                dh_all[:].rearrange("p (g o) -> p g o", o=65)[:, :, 64:65], 1.0)

            # ---------- setup: squared norms + h|dW records ----------
            with tc.tile_pool(name="su", bufs=1) as su, \
                 tc.tile_pool(name="su2", bufs=2) as su2, \
                 tc.tile_pool(name="sup", bufs=2, space="PSUM") as sup:
                ones3 = su.tile([3, 1], f32, tag="ones3")
                nc.vector.memset(ones3[:], 1.0)
                onesr = su.tile([1, rn], f32, tag="onesr")
                nc.vector.memset(onesr[:], 1.0)
                nsqr = su.tile([1, rn], f32, tag="nsqr")
                q3sq = su.tile([3, rn], f32, tag="q3sq")
                nc.scalar.activation(out=q3sq[:], in_=B5[0:3, :], func=AF.Square)
                for i in range(rn // 512):
                    pssq = sup.tile([1, 512], f32, tag="pssq", space="PSUM")
                    nc.tensor.matmul(out=pssq[:], lhsT=ones3[:],
                                     rhs=q3sq[:, i * 512:(i + 1) * 512],
                                     start=True, stop=True)
                    nc.scalar.mul(out=nsqr[:, i * 512:(i + 1) * 512], in_=pssq[:],
                                  mul=-1.0)
                nc.sync.dma_start(B5[3:4, :], onesr[:])
                nc.sync.dma_start(B5[4:5, :], nsqr[:])
                # query side
                nsqq = su.tile([1, half], f32, tag="nsqq")
                qqsq = su.tile([3, half], f32, tag="qqsq")
                nc.scalar.activation(out=qqsq[:], in_=q3q_sb[:], func=AF.Square)
                for i in range(half // 512):
                    psq = sup.tile([1, 512], f32, tag="pssq", space="PSUM")
                    nc.tensor.matmul(out=psq[:], lhsT=ones3[:],
                                     rhs=qqsq[:, i * 512:(i + 1) * 512],
                                     start=True, stop=True)
                    nc.scalar.mul(out=nsqq[:, i * 512:(i + 1) * 512], in_=psq[:],
                                  mul=-1.0)
                nc.scalar.mul(out=A5q[0:3, :], in_=q3q_sb[:], mul=2.0)
                nc.sync.dma_start(A5q[3:4, :], nsqq[:])
                nc.sync.dma_start(A5q[4:5, :], onesr[:, 0:half])
                for i in range(nsc):
                    sl = slice(i * 128, (i + 1) * 128)
                    psh = sup.tile([128, 80], f32, tag="psh", space="PSUM")
                    nc.tensor.matmul(out=psh[:], lhsT=B5[0:3, sl],
                                     rhs=w1ww_sb[:], start=True, stop=True)
                    hsb = su2.tile([128, 80], f32, tag="hsb")
                    nc.scalar.copy(out=hsb[:], in_=psh[:])
                    nc.sync.dma_start(recs[sl, 0:80], hsb[:])

            # ---------- phase A + B1 ----------
            with tc.tile_pool(name="a1", bufs=1) as a1, \
                 tc.tile_pool(name="a2", bufs=2) as a2, \
                 tc.tile_pool(name="ap2", bufs=2, space="PSUM") as ap2:
                for ci in range(nch):
                    qsl = slice(ci * 128, (ci + 1) * 128)
                    vals = a1.tile([128, rn], f32, tag="vals")
                    qw = min(1024, rn)
                    for qd in range(rn // qw):
                        psd = ap2.tile([128, qw], f32, tag="psd", space="PSUM")
                        for hh in range(qw // 512):
                            nc.tensor.matmul(
                                out=psd[:, hh * 512:(hh + 1) * 512], lhsT=A5q[:, qsl],
                                rhs=B5[:, qd * qw + hh * 512:qd * qw + (hh + 1) * 512],
                                start=True, stop=True)
                        nc.scalar.copy(out=vals[:, qd * qw:qd * qw + 512],
                                       in_=psd[:, 0:512])
                        if qw > 512:
                            nc.scalar.copy(out=vals[:, qd * qw + 512:(qd + 1) * qw],
                                           in_=psd[:, 512:1024])
                    nseg = 16
                    sv = a2.tile([128, nseg * 8], f32, tag="sv")
                    for sgi in range(nseg):
                        nc.vector.max(out=sv[:, sgi * 8:(sgi + 1) * 8],
                                      in_=vals[:, sgi * (rn // 16):(sgi + 1) * (rn // 16)])
                    m1 = a2.tile([128, 8], f32, tag="m1")
                    m2 = a2.tile([128, 8], f32, tag="m2")
                    sv2 = a2.tile([128, nseg * 8], f32, tag="sv2")
                    nc.vector.max(out=m1[:], in_=sv[:])
                    nc.vector.match_replace(out=sv2[:], in_to_replace=m1[:],
                                            in_values=sv[:], imm_value=-1e30)
                    nc.vector.max(out=m2[:], in_=sv2[:])
                    nc.vector.max_index(out=idx_all[:, ci * K:ci * K + 8],
                                        in_max=m1[:], in_values=vals[:])
                    nc.vector.max_index(out=idx_all[:, ci * K + 8:ci * K + 16],
                                        in_max=m2[:], in_values=vals[:])

                    # B1: gather records, delta-h, moments
                    G = a2.tile([128, K, REC], f32, tag="G")
                    for k in range(K):
                        nc.gpsimd.indirect_dma_start(
                            out=G[:, k, :], out_offset=None, in_=recs[:],
                            in_offset=bass.IndirectOffsetOnAxis(
                                ap=idx_all[:, ci * K + k:ci * K + k + 1], axis=0))
                    psh = ap2.tile([128, 80], f32, tag="psh2", space="PSUM")
                    nc.tensor.matmul(out=psh[:], lhsT=q3q_sb[:, qsl],
                                     rhs=w1ww_sb[:], start=True, stop=True)
                    hq = a2.tile([128, 80], f32, tag="hq")
                    nc.scalar.copy(out=hq[:], in_=psh[:])
                    dh_ci = dh_all[:, ci * K * 65:(ci + 1) * K * 65].rearrange(
                        "p (k j) -> p k j", k=K)[:, :, 0:64]
                    nc.vector.tensor_sub(out=dh_ci, in0=G[:, :, 0:64],
                                         in1=hq[:, 0:64].rearrange("p (o j) -> p o j", o=1).broadcast_to([128, K, 64]))
                    Gflat = G[:].rearrange("p k r -> p (k r)")
                    nc.vector.tensor_sub(out=wdiff_all[:, ci * K:(ci + 1) * K],
                                         in0=Gflat[:, 64:64 + 129 * (K - 1) + 1:129],
                                         in1=hq[:, 64:80])
                    for k in range(K):
                        base = ci * K * 65 + k * 65
                        dsl = dh_all[:, base:base + 64]
                        dsl65 = dh_all[:, base:base + 65]
                        st = (ci == 0 and k == 0)
                        sp = (ci == nch - 1 and k == K - 1)
                        nc.tensor.matmul(out=ps_mh[:], lhsT=dsl, rhs=dsl65,
                                         start=st, stop=sp, skip_group_check=True)

            # ---------- AR1 + BN1/BN3 stat folding + r1 ----------
            with tc.tile_pool(name="st", bufs=1) as st, \
                 tc.tile_pool(name="stp", bufs=2, space="PSUM") as stp:
                mh_sb = st.tile([64, 65], f32)
                nc.scalar.copy(out=mh_sb[:], in_=ps_mh[:])
                nc.sync.dma_start(ar1i[:], mh_sb[:])
                nc.gpsimd.collective_compute(
                    "AllReduce", mybir.AluOpType.add,
                    ins=[ar1i[:]], outs=[ar1o[:]], replica_groups=rg)
                nc.sync.dma_start(mh_g[:], ar1o[:])

                mud = st.tile([64, 1], f32)
                nc.vector.tensor_scalar_mul(mud[:], mh_g[:, 64:65], 1.0 / ntot)
                mask = st.tile([64, 64], f32)
                nc.vector.tensor_mul(out=mask[:], in0=mh_g[:, 0:64],
                                     in1=ident[0:64, 0:64])
                psd1 = stp.tile([64, 1], f32, tag="stsc", space="PSUM")
                nc.tensor.matmul(out=psd1[:], lhsT=mask[:], rhs=ones128[0:64, :],
                                 start=True, stop=True)
                var1 = st.tile([64, 1], f32)
                nc.scalar.mul(out=var1[:], in_=psd1[:], mul=1.0 / ntot)
                musq = st.tile([64, 1], f32)
                nc.scalar.activation(out=musq[:], in_=mud[:], func=AF.Square)
                nc.vector.tensor_sub(out=var1[:], in0=var1[:], in1=musq[:])
                rs1 = st.tile([64, 1], f32)
                nc.vector.tensor_scalar_add(var1[:], var1[:], EPS)
                nc.scalar.activation(out=rs1[:], in_=var1[:], func=AF.Sqrt)
                nc.vector.reciprocal(out=rs1[:], in_=rs1[:])
                nc.vector.tensor_mul(out=s1[:], in0=rs1[:], in1=g1c)
                inv1 = st.tile([64, 1], f32)
                nc.vector.reciprocal(out=inv1[:], in_=s1[:])
                nc.vector.tensor_mul(out=inv1[:], in0=inv1[:], in1=be1c)
                nc.vector.tensor_sub(out=c1[:], in0=inv1[:], in1=mud[:])

                # BN3 via pinv: M3 = G Mh G^T
                psp1 = stp.tile([3, 64], f32, tag="stsc", space="PSUM")
                nc.tensor.matmul(out=psp1[:], lhsT=gpv_sb, rhs=mh_g[:, 0:64],
                                 start=True, stop=True)
                p1 = st.tile([3, 64], f32)
                nc.scalar.copy(out=p1[:], in_=psp1[:])
                psp1t = stp.tile([64, 3], f32, tag="stsc", space="PSUM")
                nc.tensor.matmul(out=psp1t[:], lhsT=p1[:], rhs=ident[0:3, 0:3],
                                 is_transpose=True, start=True, stop=True)
                p1t = st.tile([64, 3], f32)
                nc.scalar.copy(out=p1t[:], in_=psp1t[:])
                psm3 = stp.tile([3, 3], f32, tag="stsc", space="PSUM")
                nc.tensor.matmul(out=psm3[:], lhsT=p1t[:], rhs=gpv_sb,
                                 start=True, stop=True)
                m3 = st.tile([3, 3], f32)
                nc.scalar.mul(out=m3[:], in_=psm3[:], mul=1.0 / ntot)
                psmu3 = stp.tile([3, 1], f32, tag="stsc", space="PSUM")
                nc.tensor.matmul(out=psmu3[:], lhsT=gpv_sb, rhs=mud[:],
                                 start=True, stop=True)
                mu3 = st.tile([3, 1], f32)
                nc.scalar.copy(out=mu3[:], in_=psmu3[:])
                psm3r = stp.tile([1, 3], f32, tag="stsc", space="PSUM")
                nc.tensor.matmul(out=psm3r[:], lhsT=mu3[:], rhs=ident[0:3, 0:3],
                                 is_transpose=True, start=True, stop=True)
                mu3r = st.tile([1, 3], f32)
                nc.scalar.copy(out=mu3r[:], in_=psm3r[:])
                pso3 = stp.tile([3, 3], f32, tag="stsc", space="PSUM")
                nc.tensor.matmul(out=pso3[:], lhsT=mu3r[:], rhs=mu3r[:],
                                 start=True, stop=True)
                nc.vector.tensor_sub(out=m3[:], in0=m3[:], in1=pso3[:])  # Cov3
                wwt = w1ww_sb[:, 64:80]
                psq1 = stp.tile([3, K], f32, tag="stsc", space="PSUM")
                nc.tensor.matmul(out=psq1[:], lhsT=m3[:], rhs=wwt,
                                 start=True, stop=True)
                prod = st.tile([3, K], f32)
                nc.vector.tensor_mul(out=prod[:], in0=psq1[:], in1=wwt)
                ones3b = st.tile([3, 1], f32, tag="ones3b")
                nc.vector.memset(ones3b[:], 1.0)
                psv3 = stp.tile([K, 1], f32, tag="stsc", space="PSUM")
                nc.tensor.matmul(out=psv3[:], lhsT=prod[:], rhs=ones3b[:],
                                 start=True, stop=True)
                s3 = st.tile([K, 1], f32)
                v3sb = st.tile([K, 1], f32, tag="v3sb")
                nc.vector.tensor_scalar_add(v3sb[:], psv3[:], EPS)
                nc.scalar.activation(out=s3[:], in_=v3sb[:], func=AF.Sqrt)
                nc.vector.reciprocal(out=s3[:], in_=s3[:])
                nc.vector.tensor_mul(out=s3[:], in0=s3[:], in1=gwc)
                psw3 = stp.tile([K, 1], f32, tag="stsc", space="PSUM")
                nc.tensor.matmul(out=psw3[:], lhsT=wwt, rhs=mu3[:],
                                 start=True, stop=True)
                inv3 = st.tile([K, 1], f32)
                nc.vector.reciprocal(out=inv3[:], in_=s3[:])
                nc.vector.tensor_mul(out=inv3[:], in0=inv3[:], in1=bewc)
                cc3 = st.tile([K, 1], f32)
                nc.vector.tensor_sub(out=cc3[:], in0=inv3[:], in1=psw3[:])
                psr = stp.tile([1, K], f32, tag="stsc", space="PSUM")
                s3r = st.tile([1, K], f32)
                nc.tensor.matmul(out=psr[:], lhsT=s3[:], rhs=ident[0:K, 0:K],
                                 is_transpose=True, start=True, stop=True)
                nc.scalar.copy(out=s3r[:], in_=psr[:])
                psr2 = stp.tile([1, K], f32, tag="stsc", space="PSUM")
                cc3r = st.tile([1, K], f32)
                nc.tensor.matmul(out=psr2[:], lhsT=cc3[:], rhs=ident[0:K, 0:K],
                                 is_transpose=True, start=True, stop=True)
                nc.scalar.copy(out=cc3r[:], in_=psr2[:])
                s3rep = st.tile([128, K], f32)
                nc.gpsimd.partition_broadcast(s3rep[:], s3r[:])
                cc3rep = st.tile([128, K], f32)
                nc.gpsimd.partition_broadcast(cc3rep[:], cc3r[:])
                nc.vector.tensor_add(
                    out=wdp_all[:],
                    in0=wdiff_all[:],
                    in1=cc3rep[:].rearrange("p (o k) -> p o k", o=1).broadcast_to([128, nch, K]))
                nc.scalar.activation(out=wdp_all[:], in_=wdp_all[:], func=AF.Relu)
                nc.vector.tensor_mul(
                    out=wdp_all[:], in0=wdp_all[:],
                    in1=s3rep[:].rearrange("p (o k) -> p o k", o=1).broadcast_to([128, nch, K]))
                nc.sync.dma_start(wdpo[:], wdp_all[:])
                nc.sync.dma_start(idxo[:], idx_all[:])
                nc.sync.dma_start(s1o[:], s1[:])

            # ---------- r1 = relu(dh^T + c1) -> DRAM ----------
            with tc.tile_pool(name="r1g", bufs=3) as r1g, \
                 tc.tile_pool(name="r1p", bufs=2, space="PSUM") as r1p:
                for ci in range(nch):
                    for grp in range(4):
                        psdht = r1p.tile([64, 512], f32, tag="psdht", space="PSUM")
                        for k2 in range(4):
                            k = grp * 4 + k2
                            nc.tensor.matmul(
                                out=psdht[:, k2 * 128:(k2 + 1) * 128],
                                lhsT=dh_all[:, ci * K * 65 + k * 65:ci * K * 65 + k * 65 + 64],
                                rhs=ident[:], is_transpose=True, start=True, stop=True)
                        r1t = r1g.tile([64, 512], bf, tag="r1t")
                        nc.scalar.activation(out=r1t[:], in_=psdht[:],
                                             func=AF.Relu, bias=c1[:])
                        nc.sync.dma_start(
                            r1o[:, ci * 2048 + grp * 512:ci * 2048 + (grp + 1) * 512],
                            r1t[:])

    nc.finalize()
    return nc


def build_main(rn=4096, n_cores=N_CORES):
    half = rn // 2
    nch = half // 128
    nsc = rn // 128

    nc = bacc.Bacc("TRN2", target_bir_lowering=False, debug=False,
                   num_devices=n_cores, enable_asserts=False)

    fe8 = nc.dram_tensor("fe8", [C, rn], mybir.dt.int8, kind="ExternalInput").ap()
    # wp2[128,196]: [:,0:128]=W2a.T; [:,128:192]=W2b.T transposed ([C,64]);
    # [:,192]=g2; [:,193]=be2; [:,194]=int8 dequant scale per channel
    wp2 = nc.dram_tensor("wp2", [128, 196], f16, kind="ExternalInput").ap()
    idxi = nc.dram_tensor("idxi", [128, nch * K], u32, kind="ExternalInput").ap()
    r1i = nc.dram_tensor("r1i", [64, nch * K * 128], bf, kind="ExternalInput").ap()
    wdpi = nc.dram_tensor("wdpi", [128, nch * K], f32, kind="ExternalInput").ap()
    s1i = nc.dram_tensor("s1i", [64, 1], f32, kind="ExternalInput").ap()

    y16 = nc.dram_tensor("y16", [C, half], f16, kind="ExternalOutput").ap()

    garr = nc.dram_tensor("garr", [rn, C], bf).ap()
    ar2i = nc.dram_tensor("ar2i", [C, 2], f32).ap()
    ar2o = nc.dram_tensor("ar2o", [C, 2], f32, addr_space="Shared").ap()
    rg = [list(range(n_cores))]

    with tile.TileContext(nc) as tc:
        with tc.tile_pool(name="persist", bufs=1) as pp, \
             tc.tile_pool(name="ppsum", bufs=1, space="PSUM") as ppp:
            ident = pp.tile([128, 128], f32)
            make_identity(nc, ident[:])
            ident_bf = pp.tile([128, 128], bf)
            nc.vector.tensor_copy(out=ident_bf[:], in_=ident[:])
            onesrow_bf = pp.tile([1, 128], bf)
            nc.vector.memset(onesrow_bf[:], 1.0)

            fe8_sb = pp.tile([C, rn], mybir.dt.int8)
            nc.sync.dma_start(fe8_sb[:], fe8[:])
            fe_sb = pp.tile([C, rn], f16)
            nc.vector.tensor_copy(out=fe_sb[:], in_=fe8_sb[:])
            wp2_sb16 = pp.tile([128, 196], f16)
            nc.sync.dma_start(wp2_sb16[:], wp2[:])
            wp2_sb = pp.tile([128, 196], f32)
            nc.vector.tensor_copy(out=wp2_sb[:], in_=wp2_sb16[:])
            idx_sb = pp.tile([128, nch * K], u32)
            nc.sync.dma_start(idx_sb[:], idxi[:])
            wdp_sb = pp.tile([128, nch * K], f32)
            nc.sync.dma_start(wdp_sb[:], wdpi[:])
            s1 = pp.tile([64, 1], f32)
            nc.sync.dma_start(s1[:], s1i[:])
            r1_sb = pp.tile([64, nch * K * 128], bf)
            nc.sync.dma_start(r1_sb[:], r1i[:])

            # fold int8 dequant scale into W2a rows
            w2at_f = pp.tile([128, 128], f32)
            nc.vector.tensor_mul(out=w2at_f[:], in0=wp2_sb[:, 0:128],
                                 in1=wp2_sb[:, 194:195].broadcast_to([128, 128]))
            w2at16 = pp.tile([128, 128], f16)
            nc.scalar.copy(out=w2at16[:], in_=w2at_f[:])
            g2c = wp2_sb[:, 192:193]
            be2c = wp2_sb[:, 193:194]

            G_all = pp.tile([128, nch * K * C], bf)
            w2bt = pp.tile([64, C], f32)
            w2bt1 = pp.tile([64, C], f32)
            w2bt1_bf = pp.tile([64, C], bf)
            w2bt2 = pp.tile([64, C], f32)
            w2bt2_bf = pp.tile([64, C], bf)
            c2row = pp.tile([1, C], f32)
            c2row_bf = pp.tile([1, C], bf)
            s2rep = pp.tile([C, C], f32)
            s2rep_bf = pp.tile([C, C], bf)
            bn_all = pp.tile([128, nch * 4 * 6], f32)

            pswt = ppp.tile([64, 128], f32, space="PSUM")
            nc.tensor.matmul(out=pswt[:], lhsT=wp2_sb[:, 128:192], rhs=ident[:],
                             is_transpose=True, start=True, stop=True)
            nc.scalar.copy(out=w2bt[:], in_=pswt[:])
            nc.vector.tensor_mul(out=w2bt1[:], in0=w2bt[:],
                                 in1=s1[:].broadcast_to([64, C]))
            nc.scalar.copy(out=w2bt1_bf[:], in_=w2bt1[:])

            # ---------- garr = W2a^T fe ----------
            with tc.tile_pool(name="su2", bufs=2) as su2, \
                 tc.tile_pool(name="sup", bufs=2, space="PSUM") as sup:
                for i in range(nsc):
                    sl = slice(i * 128, (i + 1) * 128)
                    psg = sup.tile([128, C], f32, tag="psg", space="PSUM")
                    nc.tensor.matmul(out=psg[:], lhsT=fe_sb[:, sl],
                                     rhs=w2at16[:], start=True, stop=True)
                    gsb = su2.tile([128, C], bf, tag="gsb")
                    nc.scalar.copy(out=gsb[:], in_=psg[:])
                    nc.sync.dma_start(garr[sl, :], gsb[:])

            # ---------- phase B2: BN2 stats ----------
            with tc.tile_pool(name="b2p", bufs=2, space="PSUM") as b2p:
                for ci in range(nch):
                    G2 = G_all[:, ci * K * C:(ci + 1) * K * C].rearrange(
                        "p (k c) -> p k c", k=K)
                    for k in range(K):
                        nc.gpsimd.indirect_dma_start(
                            out=G2[:, k, :], out_offset=None, in_=garr[:],
                            in_offset=bass.IndirectOffsetOnAxis(
                                ap=idx_sb[:, ci * K + k:ci * K + k + 1], axis=0))
                    for grp in range(4):
                        psxt = b2p.tile([128, 512], f32, tag="psxt", space="PSUM")
                        nc.tensor.matmul(
                            out=psxt[:], lhsT=w2bt1_bf[:],
                            rhs=r1_sb[:, ci * 2048 + grp * 512:ci * 2048 + (grp + 1) * 512],
                            start=True, stop=False, skip_group_check=True)
                        for k2 in range(4):
                            k = grp * 4 + k2
                            nc.tensor.matmul(
                                out=psxt[:, k2 * 128:(k2 + 1) * 128],
                                lhsT=G2[:, k, :], rhs=ident_bf[:],
                                start=False, stop=(k2 == 3), skip_group_check=True)
                        nc.vector.bn_stats(
                            out=bn_all[:, (ci * 4 + grp) * 6:(ci * 4 + grp + 1) * 6],
                            in_=psxt[:])

            # ---------- AR2 + BN2 folding + garr rescale ----------
            with tc.tile_pool(name="s2t", bufs=1) as s2t, \
                 tc.tile_pool(name="s2p", bufs=2, space="PSUM") as s2p:
                bnag = s2t.tile([128, 2], f32)
                nc.vector.bn_aggr(out=bnag[:],
                                  in_=bn_all[:].rearrange("p (g s) -> p g s", s=6))
                pay = s2t.tile([128, 2], f32)
                nc.vector.tensor_copy(out=pay[:, 0:1], in_=bnag[:, 0:1])
                msq = s2t.tile([128, 1], f32)
                nc.scalar.activation(out=msq[:], in_=bnag[:, 0:1], func=AF.Square)
                nc.vector.tensor_add(out=pay[:, 1:2], in0=bnag[:, 1:2], in1=msq[:])
                nc.sync.dma_start(ar2i[:], pay[:])
                nc.gpsimd.collective_compute(
                    "AllReduce", mybir.AluOpType.add,
                    ins=[ar2i[:]], outs=[ar2o[:]], replica_groups=rg)
                arg = s2t.tile([128, 2], f32)
                nc.sync.dma_start(arg[:], ar2o[:])
                mux = s2t.tile([128, 1], f32)
                nc.vector.tensor_scalar_mul(mux[:], arg[:, 0:1], 1.0 / n_cores)
                ex2 = s2t.tile([128, 1], f32)
                nc.vector.tensor_scalar_mul(ex2[:], arg[:, 1:2], 1.0 / n_cores)
                mxs = s2t.tile([128, 1], f32)
                nc.scalar.activation(out=mxs[:], in_=mux[:], func=AF.Square)
                varx = s2t.tile([128, 1], f32)
                nc.vector.tensor_sub(out=varx[:], in0=ex2[:], in1=mxs[:])
                s2v = s2t.tile([128, 1], f32)
                nc.vector.tensor_scalar_add(varx[:], varx[:], EPS)
                nc.scalar.activation(out=s2v[:], in_=varx[:], func=AF.Sqrt)
                nc.vector.reciprocal(out=s2v[:], in_=s2v[:])
                nc.vector.tensor_mul(out=s2v[:], in0=s2v[:], in1=g2c)
                c2p = s2t.tile([128, 1], f32)
                nc.vector.tensor_mul(out=c2p[:], in0=mux[:], in1=s2v[:])
                nc.vector.tensor_sub(out=c2p[:], in0=be2c, in1=c2p[:])
                psr3 = s2p.tile([1, 128], f32, tag="s2sc", space="PSUM")
                nc.tensor.matmul(out=psr3[:], lhsT=s2v[:], rhs=ident[:],
                                 is_transpose=True, start=True, stop=True)
                s2row = s2t.tile([1, 128], f32)
                nc.scalar.copy(out=s2row[:], in_=psr3[:])
                psr4 = s2p.tile([1, 128], f32, tag="s2sc", space="PSUM")
                nc.tensor.matmul(out=psr4[:], lhsT=c2p[:], rhs=ident[:],
                                 is_transpose=True, start=True, stop=True)
                nc.scalar.copy(out=c2row[:], in_=psr4[:])
                nc.gpsimd.partition_broadcast(s2rep[:], s2row[:])
                s2rep64 = s2t.tile([64, C], f32)
                nc.gpsimd.partition_broadcast(s2rep64[:], s2row[:])
                nc.vector.tensor_mul(out=w2bt2[:], in0=w2bt1[:], in1=s2rep64[:])
                nc.scalar.copy(out=w2bt2_bf[:], in_=w2bt2[:])
                nc.scalar.copy(out=c2row_bf[:], in_=c2row[:])
                nc.scalar.copy(out=s2rep_bf[:], in_=s2rep[:])

            # ---------- phase C ----------
            with tc.tile_pool(name="c1p", bufs=2) as cp, \
                 tc.tile_pool(name="cpp", bufs=2, space="PSUM") as cpp, \
                 tc.tile_pool(name="cop", bufs=3) as cop:
                for ci in range(nch):
                    G3 = G_all[:, ci * K * C:(ci + 1) * K * C].rearrange(
                        "p (k c) -> p k c", k=K)
                    nc.vector.tensor_mul(
                        out=G3, in0=G3,
                        in1=s2rep_bf[:].rearrange("p (o c) -> p o c", o=1).broadcast_to(
                            [128, K, C]))
                    psot = cpp.tile([128, 128], f32, tag="psot", space="PSUM")
                    for grp in range(4):
                        psz = cpp.tile([128, 512], f32, tag="psz", space="PSUM")
                        nc.tensor.matmul(
                            out=psz[:], lhsT=ident_bf[:],
                            rhs=G3[:, grp * 4:(grp + 1) * 4, :].rearrange(
                                "p k c -> p (k c)"),
                            start=True, stop=False, skip_group_check=True)
                        nc.tensor.matmul(
                            out=psz[:], lhsT=onesrow_bf[:],
                            rhs=c2row_bf[:].rearrange("o (d c) -> o d c", d=1).broadcast_to(
                                [1, 4, C]),
                            start=False, stop=False, skip_group_check=True)
                        for k2 in range(4):
                            k = grp * 4 + k2
                            zsl = psz[:, k2 * 128:(k2 + 1) * 128]
                            nc.tensor.matmul(
                                out=zsl,
                                lhsT=r1_sb[:, ci * 2048 + k * 128:ci * 2048 + (k + 1) * 128],
                                rhs=w2bt2_bf[:], start=False,
                                stop=(k2 == 3),
                                skip_group_check=True)
                            ek = cp.tile([128, 128], f32, tag="ek")
                            nc.scalar.activation(
                                out=ek[:], in_=zsl, func=AF.Relu,
                                scale=wdp_sb[:, ci * K + k:ci * K + k + 1])
                            nc.tensor.matmul(out=psot[:], lhsT=ek[:], rhs=ident[:],
                                             is_transpose=True, start=(k == 0),
                                             stop=(k == K - 1), skip_group_check=True)
                    osb = cop.tile([128, 128], f16, tag="osb")
                    nc.scalar.copy(out=osb[:], in_=psot[:])
                    nc.sync.dma_start(y16[:, ci * 128:(ci + 1) * 128], osb[:])

    nc.finalize()
    return nc


_RUNNERS = None


def _make_runner(nc, n_cores):
    import jax
    from jax.experimental.shard_map import shard_map
    from jax.sharding import Mesh, PartitionSpec
    from concourse import bass2jax, mybir as mb
    from concourse.bass2jax import partition_id_tensor

    partition_name = nc.partition_id_tensor.name if nc.partition_id_tensor else None
    in_names, out_names, out_avals = [], [], []
    for alloc in nc.m.functions[0].allocations:
        if not isinstance(alloc, mb.MemoryLocationSet):
            continue
        name = alloc.memorylocations[0].name
        if alloc.kind == "ExternalInput":
            if name != partition_name:
                in_names.append(name)
        elif alloc.kind == "ExternalOutput":
            shape = tuple(alloc.tensor_shape)
            dtype = mb.dt.np(alloc.dtype)
            out_names.append(name)
            out_avals.append(jax.core.ShapedArray(shape, dtype))
    n_params = len(in_names)
    all_in_names = list(in_names) + list(out_names)
    if partition_name is not None:
        all_in_names.append(partition_name)

    def _body(*args):
        operands = list(args)
        if partition_name is not None:
            operands.append(partition_id_tensor())
        outs = bass2jax._bass_exec_p.bind(
            *operands,
            out_avals=tuple(out_avals),
            in_names=tuple(all_in_names),
            out_names=tuple(out_names),
            lowering_input_output_aliases=(),
            sim_require_finite=True,
            sim_require_nnan=True,
            nc=nc,
        )
        return tuple(outs)

    import numpy as _np
    devices = jax.devices()[:n_cores]
    mesh = Mesh(_np.asarray(devices), ("core",))
    n_outs = len(out_names)
    sharded = jax.jit(
        shard_map(_body, mesh=mesh,
                  in_specs=(PartitionSpec("core"),) * (n_params + n_outs),
                  out_specs=(PartitionSpec("core"),) * n_outs,
                  check_rep=False),
        keep_unused=True)
    return dict(fn=sharded, in_names=in_names, out_names=out_names,
                out_avals=out_avals, mesh=mesh)


def _get_runners(rn):
    global _RUNNERS
    if _RUNNERS is not None:
        return _RUNNERS
    import jax
    import jax.numpy as jnp
    from jax.sharding import NamedSharding, PartitionSpec
    from concourse.bass2jax import install_neuronx_cc_hook
    install_neuronx_cc_hook()
    r1 = _make_runner(build_knn(rn), N_CORES)
    r2 = _make_runner(build_main(rn), N_CORES)
    shd = NamedSharding(r1["mesh"], PartitionSpec("core"))
    # dummy output buffers (kernels fully overwrite outputs; reused each call)
    for r in (r1, r2):
        dummies = []
        for av in r["out_avals"]:
            dummies.append(jnp.zeros((N_CORES * av.shape[0], *av.shape[1:]),
                                     av.dtype, device=shd))
        jax.block_until_ready(dummies)
        r["dummies"] = dummies
    _RUNNERS = (r1, r2)
    return _RUNNERS


def kernel(**inputs):
    F_E = np.asarray(inputs["F_E"], dtype=np.float32)
    Q = np.asarray(inputs["Q_prime"], dtype=np.float32)
    rn = F_E.shape[2]
    half = rn // 2
    r1, r2 = _get_runners(rn)

    # int8 feature quantization in background threads (overlaps call1 dispatch)
    fe8_st = np.empty((B, 2, C, rn), np.int8)
    scale = np.empty((B, C), np.float32)

    def quant(b, lo, hi):
        amax = np.abs(F_E[b, lo:hi]).max(axis=1)
        s = np.maximum(amax, 1e-30) / 127.0
        scale[b, lo:hi] = s
        q = np.rint(F_E[b, lo:hi] * (1.0 / s)[:, None]).astype(np.int8)
        fe8_st[b, 0, lo:hi] = q
        fe8_st[b, 1, lo:hi] = q
    qfuts = [_POOL.submit(quant, b, lo, lo + 64)
             for b in range(B) for lo in (0, 64)]

    # --- small host prep (query side + packed weights) ---
    q3c_st = np.ascontiguousarray(np.repeat(Q, 2, axis=0)).reshape(2 * B * 3, rn)
    q3q_st = np.ascontiguousarray(
        Q.reshape(B, 3, 2, half).transpose(0, 2, 1, 3)).reshape(2 * B * 3, half)
    W1 = np.asarray(inputs["W1"], np.float32)
    Ww = np.asarray(inputs["Ww"], np.float32)
    wp1 = np.zeros((64, 88), np.float32)
    wp1[0:3, 0:80] = np.concatenate([W1.T, Ww.T], axis=1)
    wp1[:, 80:83] = np.linalg.pinv(W1).T.astype(np.float32)
    wp1[:, 83] = np.asarray(inputs["g1"], np.float32)
    wp1[:, 84] = np.asarray(inputs["be1"], np.float32)
    wp1[0:K, 85] = np.asarray(inputs["gw"], np.float32)
    wp1[0:K, 86] = np.asarray(inputs["bew"], np.float32)
    wp1_st = np.broadcast_to(wp1, (N_CORES, 64, 88)).reshape(N_CORES * 64, 88)

    args1 = dict(q3c=q3c_st, q3q=q3q_st, wp1=wp1_st)
    out1 = r1["fn"](*[args1[nm] for nm in r1["in_names"]], *r1["dummies"])
    state = dict(zip(r1["out_names"], out1))

    # --- big feature upload dispatched while call1 is in flight ---
    W2 = np.asarray(inputs["W2"], np.float32)
    wp2 = np.zeros((B, 128, 196), np.float16)
    wp2[:, :, 0:128] = W2[:, :C].T.astype(np.float16)
    wp2[:, :, 128:192] = W2[:, C:].astype(np.float16)   # (C,64) = (W2b^T)^T
    wp2[:, :, 192] = np.asarray(inputs["g2"], np.float32).astype(np.float16)
    wp2[:, :, 193] = np.asarray(inputs["be2"], np.float32).astype(np.float16)
    for f in qfuts:
        f.result()
    wp2[:, :, 194] = scale.astype(np.float16)
    wp2_st = np.repeat(wp2, 2, axis=0).reshape(N_CORES * 128, 196)

    args2 = dict(fe8=fe8_st.reshape(2 * B * C, rn), wp2=wp2_st,
                 idxi=state["idxo"], r1i=state["r1o"], wdpi=state["wdpo"],
                 s1i=state["s1o"])
    out2 = r2["fn"](*[args2[nm] for nm in r2["in_names"]], *r2["dummies"])
    yg = out2[r2["out_names"].index("y16")]  # (2*B*C, half) f16, sharded

    res = np.empty((B, C, rn), np.float32)

    def fetch_add(shard):
        data = np.asarray(shard.data)            # (C, half) f16
        row0 = shard.index[0].start or 0
        c = row0 // C
        b, h = c // 2, c % 2
        np.add(F_E[b, :, h * half:(h + 1) * half], data,
               out=res[b, :, h * half:(h + 1) * half])
    list(_POOL.map(fetch_add, yg.addressable_shards))
    return res
